# revision 3
# baseline (speedup 1.0000x reference)
"""Trainium2 Bass kernel for CLIP attention pooling.

Reference computation (N=4096, D=1024, fp32):
    q = x @ Wq.T + bq
    k = x @ Wk.T + bk
    attn = softmax(q @ k.T, axis=-1)
    out = attn @ x

Math notes:
  * scores = q @ k.T = q @ Wk @ x.T + (q.bk) 1^T. The (q.bk) term is
    constant along the softmax axis, so bk never needs to be computed.
  * q @ Wk = x @ (Wq.T @ Wk) + bq @ Wk: both projections fold into one
    matrix M = Wq.T @ Wk and a row c = bq @ Wk (host-precomputed).
  * Per core (512 query rows):
        tT = M^T . xs^T + c          [D, 512]   (transposed layout)
        S  = t . x^T                 [512, 4096]
        P  = softmax(S)              (online, running-max)
        out = P @ x                  [512, 1024]

Schedule (v3):
  * phase A: M chunks stream on the sync HWDGE ring, xs chunks on the
    scalar ring (independent trigger FIFOs); e-outer over 8 PSUM banks;
    the bias row enters via a K=1 (c x ones) matmul per bank. The
    phase-B chunk-0 stream DMA is interleaved into the M trigger
    sequence so it lands just before phase A's compute finishes.
  * phase B: x^T streams in 512-column chunks (double-buffered).
    Softmax is ONLINE: per (i, chunk) a running negated max is
    maintained on DVE straight out of PSUM, and ACT applies
    exp(PSUM - runmax) directly into bf16 E with accum_out collecting
    per-chunk partial sums. No S buffer exists.
  * after the last chunk per i: c_k = exp(m_k - m_final), Z = sum_k
    z_k c_k, g_k = c_k / Z. The g_k become 32 per-(i,chunk) DIAGONAL
    matrices (bf16), built i-grouped alternating DVE/ACT; phase C
    "transposes" are plain matmuls E_tile @ diag(g) so the softmax
    normalization and running-max corrections ride the mandatory
    transpose for free.
  * phase C: x (bf16) is fully resident in SBUF (8MB, loaded on the
    sync ring behind the phase-B stream; its buffer aliases only the
    phase-A weight pool so the triggers fire as soon as the ring
    drains). Passes over i-tiles {0,1}/{2}/{3}: pass 0 needs only the
    first two diags so its transposes start while phase B's last
    matmuls still run; the last pass leaves just 512KB of output for
    the tail. A single shared PSUM transpose pool avoids cross-pass
    bank WARs (pass 1 lands on never-used banks). Output copy/DMA
    pairs alternate DVE/ACT engines and sync/scalar DMA rings.
"""

import os
from contextlib import ExitStack

import numpy as np
import ml_dtypes

import concourse.bass as bass
import concourse.mybir as mybir
import concourse.tile as tile
from concourse import bacc
from concourse.bass_utils import run_bass_kernel_spmd
from concourse.masks import make_identity

N, D = 4096, 1024
NCORES = 8
R = N // NCORES  # 512 query rows per core
PT = 128  # partition tile
EC = D // PT  # 8 contraction chunks of the model dim
IT = R // PT  # 4 query tiles per core
JC = N // 512  # 8 key chunks of 512
JT = N // PT  # 32 key tiles of 128

F32 = mybir.dt.float32
F32R = mybir.dt.float32r
BF16 = mybir.dt.bfloat16
AX = mybir.AxisListType
AF = mybir.ActivationFunctionType
ALU = mybir.AluOpType

PASSES = ((0, 1), (2,), (3,))


def _emit(nc: bass.Bass, tc: tile.TileContext, aps: dict):
    xTb, xTs, mw, cw, ones, xb, out = (
        aps["xTb"], aps["xTs"], aps["mw"], aps["cw"],
        aps["ones"], aps["xb"], aps["out"],
    )

    with ExitStack() as big:
        persist = big.enter_context(tc.tile_pool(name="persist", bufs=1))

        ident = persist.tile([PT, PT], BF16)
        make_identity(nc, ident)
        c_sb = persist.tile([1, D], F32R)
        ones_sb = persist.tile([1, R], F32R)

        tT_sb = persist.tile([PT, EC, R], F32R)
        E_bf = [persist.tile([PT, N], BF16, name=f"E{i}") for i in range(IT)]
        nmk = [persist.tile([PT, JC], F32, name=f"nmk{i}") for i in range(IT)]
        tmx = [persist.tile([PT, JC], F32, name=f"tmx{i}") for i in range(IT)]
        zpart = [persist.tile([PT, JC], F32, name=f"zp{i}") for i in range(IT)]
        ck = [persist.tile([PT, JC], F32, name=f"ck{i}") for i in range(IT)]
        gk = [persist.tile([PT, JC], F32, name=f"gk{i}") for i in range(IT)]
        zsum = [persist.tile([PT, 1], F32, name=f"z{i}") for i in range(IT)]
        rz = [persist.tile([PT, 1], F32, name=f"rz{i}") for i in range(IT)]
        diag = persist.tile([PT, IT, JC, PT], BF16)

        # opened before wpool so its addresses never overlap the weights;
        # the early stream triggers can then issue during phase A.
        xtpool = big.enter_context(tc.tile_pool(name="xtpool", bufs=2))
        xtjs = {}
        for j in range(JC):
            xtjs[j] = xtpool.tile([PT, EC, 512], F32R, tag="xtj", name="xtj")

        # ---- Phase A: tT = M^T.xs^T + c  (transposed layout)
        with ExitStack() as pha:
            wpool = pha.enter_context(tc.tile_pool(name="wpool", bufs=1))
            apsum = pha.enter_context(tc.tile_pool(name="apsum", bufs=1, space="PSUM"))

            m_sb = wpool.tile([PT, EC, D], F32R)
            xts_sb = wpool.tile([PT, EC, R], F32R)

            m_r = mw.rearrange("(t p) d -> p t d", p=PT)
            xTs_r = xTs.rearrange("(t p) i -> p t i", p=PT)
            # M rides the sync HWDGE ring, xs + bias the scalar ring: the
            # trigger FIFOs are independent and the SDMA engines round-robin
            # between them. Chunk 0 of the phase-B stream is slotted in
            # before the last two M chunks: phase A's compute tail covers it.
            nc.sync.dma_start(m_sb[:, 0, 0:256], m_r[:, 0, 0:256])
            nc.scalar.dma_start(xts_sb[:, 0, :], xTs_r[:, 0, :])
            nc.sync.dma_start(m_sb[:, 0, 256:D], m_r[:, 0, 256:D])
            nc.scalar.dma_start(xts_sb[:, 1, :], xTs_r[:, 1, :])
            nc.scalar.dma_start(c_sb, cw)
            nc.scalar.dma_start(ones_sb, ones)
            for e in range(1, EC - 2):
                nc.sync.dma_start(m_sb[:, e, :], m_r[:, e, :])
            nc.sync.dma_start(xtjs[0], xTb[0])
            for e in range(EC - 2, EC):
                nc.sync.dma_start(m_sb[:, e, :], m_r[:, e, :])
            for e in range(2, EC):
                nc.scalar.dma_start(xts_sb[:, e, :], xTs_r[:, e, :])

            tps = [
                apsum.tile([PT, R], F32, tag=f"tp{d}", name=f"tp{d}")
                for d in range(EC)
            ]
            for e in range(EC):
                for d in range(EC):
                    nc.tensor.matmul(
                        tps[d],
                        m_sb[:, e, d * PT : (d + 1) * PT],
                        xts_sb[:, e, :],
                        start=(e == 0),
                        stop=False,
                    )
            for d in range(EC):
                # bias row: tT[d_block, :] += c[d_block] (x) ones
                nc.tensor.matmul(
                    tps[d],
                    c_sb[:, d * PT : (d + 1) * PT],
                    ones_sb,
                    start=False,
                    stop=True,
                )
                if d % 2 == 0:
                    nc.vector.tensor_copy(tT_sb[:, d, :], tps[d])
                else:
                    nc.scalar.activation(tT_sb[:, d, :], tps[d], func=AF.Copy)

        # ---- Phase B: S chunks in PSUM + online softmax straight to E.
        with ExitStack() as phb:
            spsum = phb.enter_context(tc.tile_pool(name="spsum", bufs=5, space="PSUM"))
            for j in range(JC):
                xtj = xtjs[j]
                if j > 0:
                    nc.sync.dma_start(xtj, xTb[j])
                for i in range(IT):
                    ps = spsum.tile([PT, 512], F32, tag="Sp", name="Sp")
                    for d in range(EC):
                        nc.tensor.matmul(
                            ps,
                            tT_sb[:, d, i * PT : (i + 1) * PT],
                            xtj[:, d, :],
                            start=(d == 0),
                            stop=(d == EC - 1),
                        )
                    if j == 0:
                        nc.vector.reduce_max(
                            out=nmk[i][:, 0:1], in_=ps, axis=AX.X, negate=True
                        )
                    else:
                        nc.vector.reduce_max(
                            out=tmx[i][:, j : j + 1], in_=ps, axis=AX.X, negate=True
                        )
                        nc.vector.tensor_tensor(
                            out=nmk[i][:, j : j + 1],
                            in0=nmk[i][:, j - 1 : j],
                            in1=tmx[i][:, j : j + 1],
                            op=ALU.min,
                        )
                    nc.scalar.activation(
                        out=E_bf[i][:, j * 512 : (j + 1) * 512],
                        in_=ps,
                        func=AF.Exp,
                        bias=nmk[i][:, j : j + 1],
                        scale=1.0,
                        accum_out=zpart[i][:, j : j + 1],
                    )

            # finalize: c_k = exp(m_k - m_last), Z = sum z_k c_k, g = c_k/Z;
            # then the 8 per-chunk diag(g) tiles, i-grouped so pass 0 of
            # phase C (i-tiles 0,1) unblocks while i2/i3 still matmul.
            for i in range(IT):
                nc.scalar.activation(
                    out=ck[i],
                    in_=nmk[i],
                    func=AF.Exp,
                    bias=nmk[i][:, JC - 1 : JC],
                    scale=-1.0,
                )
                nc.vector.tensor_tensor(
                    out=gk[i], in0=zpart[i], in1=ck[i], op=ALU.mult
                )
                nc.vector.reduce_sum(out=zsum[i], in_=gk[i], axis=AX.X)
                nc.vector.reciprocal(rz[i], zsum[i])
                nc.vector.tensor_scalar_mul(gk[i], ck[i], rz[i])
                for k in range(JC):
                    if k % 2 == 0:
                        nc.vector.tensor_scalar_mul(
                            diag[:, i, k, :], ident, gk[i][:, k : k + 1]
                        )
                    else:
                        nc.scalar.activation(
                            diag[:, i, k, :],
                            ident,
                            func=AF.Copy,
                            scale=gk[i][:, k : k + 1],
                        )

        # ---- Phase C: out = P @ x with x fully resident in SBUF.
        # xb reuses the phase-A weight pool's address range; its triggers sit
        # on the sync ring behind the phase-B stream.
        xbpool = big.enter_context(tc.tile_pool(name="xbpool", bufs=1))
        xb_sb = xbpool.tile([PT, JT, D], BF16)
        xb_r = xb.rearrange("(t p) d -> p t d", p=PT)
        for g in range(8):
            nc.sync.dma_start(
                xb_sb[:, 4 * g : 4 * g + 4, :], xb_r[:, 4 * g : 4 * g + 4, :]
            )
        etpool = big.enter_context(tc.tile_pool(name="etpool", bufs=4))
        ocopy = big.enter_context(tc.tile_pool(name="ocopy", bufs=3))
        tpsum = big.enter_context(tc.tile_pool(name="tpsum", bufs=2, space="PSUM"))

        LOOK = 2
        for pi, ii in enumerate(PASSES):
            W = len(ii) * PT
            with ExitStack() as phc:
                opsum = phc.enter_context(
                    tc.tile_pool(name=f"opsum{pi}", bufs=1, space="PSUM")
                )
                oacc = {
                    (i, dn): opsum.tile(
                        [PT, 512], F32, tag=f"o{i}_{dn}", name=f"o{i}_{dn}"
                    )
                    for i in ii
                    for dn in range(2)
                }
                ets = {}
                for jtv in range(JT + LOOK):
                    if jtv < JT:
                        jt = jtv
                        # "transpose" = E_tile.T @ diag(g): per-row softmax
                        # scale applied for free by the mandatory transpose.
                        pst = tpsum.tile([PT, 2 * PT], F32, tag="tp", name="pst")
                        for kp, i in enumerate(ii):
                            nc.tensor.matmul(
                                pst[:, kp * PT : (kp + 1) * PT],
                                E_bf[i][:, jt * PT : (jt + 1) * PT],
                                diag[:, i, jt // 4, :],
                                start=True,
                                stop=True,
                                skip_group_check=True,
                            )
                        et = etpool.tile([PT, W], BF16, tag=f"et{pi}", name="et")
                        if jt % 2 == 0:
                            nc.vector.tensor_copy(et, pst[:, 0:W])
                        else:
                            nc.scalar.activation(et, pst[:, 0:W], func=AF.Copy)
                        ets[jt % 4] = et
                    if jtv >= LOOK:
                        jt = jtv - LOOK
                        for kp, i in enumerate(ii):
                            for dn in range(2):
                                nc.tensor.matmul(
                                    oacc[(i, dn)],
                                    ets[jt % 4][:, kp * PT : (kp + 1) * PT],
                                    xb_sb[:, jt, dn * 512 : (dn + 1) * 512],
                                    start=(jt == 0),
                                    stop=(jt == JT - 1),
                                )
                for kp, i in enumerate(ii):
                    for dn in range(2):
                        ot = ocopy.tile([PT, 512], F32, tag="ot", name="ot")
                        if dn == 0:
                            nc.vector.tensor_copy(ot, oacc[(i, dn)])
                            nc.sync.dma_start(
                                out[i * PT : (i + 1) * PT, 0:512], ot
                            )
                        else:
                            nc.scalar.activation(ot, oacc[(i, dn)], func=AF.Copy)
                            nc.scalar.dma_start(
                                out[i * PT : (i + 1) * PT, 512:1024], ot
                            )


def build():
    nc = bacc.Bacc(
        "TRN2",
        target_bir_lowering=False,
        debug=False,
        enable_asserts=False,
        num_devices=NCORES,
    )
    aps = {
        "xTb": nc.dram_tensor("xTb", [JC, PT, EC, 512], F32R, kind="ExternalInput").ap(),
        "xTs": nc.dram_tensor("xTs", [D, R], F32R, kind="ExternalInput").ap(),
        "mw": nc.dram_tensor("mw", [D, D], F32R, kind="ExternalInput").ap(),
        "cw": nc.dram_tensor("cw", [1, D], F32R, kind="ExternalInput").ap(),
        "ones": nc.dram_tensor("ones", [1, R], F32R, kind="ExternalInput").ap(),
        "xb": nc.dram_tensor("xb", [N, D], BF16, kind="ExternalInput").ap(),
        "out": nc.dram_tensor("out", [R, D], F32, kind="ExternalOutput").ap(),
    }
    with tile.TileContext(nc) as tc:
        _emit(nc, tc, aps)
    nc.compile()
    return nc


_NC_CACHE = None
LAST_RESULTS = None


def _get_nc():
    global _NC_CACHE
    if _NC_CACHE is None:
        _NC_CACHE = build()
    return _NC_CACHE


def make_in_maps(x, Wq, bq, Wk):
    x = np.ascontiguousarray(np.asarray(x, dtype=np.float32))
    xT = np.ascontiguousarray(x.T)
    # xTb[j, p, e, n] = xT[e*128 + p, j*512 + n]: per-(j,p) contiguous 16KB
    # blocks so the phase-B stream DMAs at full descriptor size.
    xTb = np.ascontiguousarray(
        xT.reshape(EC, PT, JC, 512).transpose(2, 1, 0, 3)
    )
    wk64 = np.asarray(Wk, dtype=np.float64)
    mw = np.ascontiguousarray(
        (np.asarray(Wq, dtype=np.float64).T @ wk64).astype(np.float32)
    )
    cw = np.ascontiguousarray(
        (np.asarray(bq, dtype=np.float64) @ wk64).astype(np.float32).reshape(1, D)
    )
    ones_arr = np.ones((1, R), dtype=np.float32)
    xb = x.astype(ml_dtypes.bfloat16)
    in_maps = []
    for c in range(NCORES):
        in_maps.append(
            {
                "xTb": xTb,
                "xTs": np.ascontiguousarray(xT[:, c * R : (c + 1) * R]),
                "mw": mw,
                "cw": cw,
                "ones": ones_arr,
                "xb": xb,
            }
        )
    return in_maps


def kernel(x, Wq, bq, Wk, bk):
    # bk only shifts each score row by a constant, which softmax cancels.
    del bk
    in_maps = make_in_maps(x, Wq, bq, Wk)
    nc = _get_nc()
    kwargs = {}
    if os.environ.get("K_TRACE_DIR"):
        import tempfile

        kwargs["tmpdir"] = tempfile.mkdtemp(dir=os.environ["K_TRACE_DIR"])
    res = run_bass_kernel_spmd(nc, in_maps, core_ids=list(range(NCORES)), **kwargs)
    global LAST_RESULTS
    LAST_RESULTS = res
    return np.concatenate(
        [np.asarray(res.results[c]["out"], dtype=np.float32) for c in range(NCORES)],
        axis=0,
    )


# revision 4
# speedup vs baseline: 1.0866x; 1.0866x over previous
"""Trainium2 Bass kernel for CLIP attention pooling.

Reference computation (N=4096, D=1024, fp32):
    q = x @ Wq.T + bq
    k = x @ Wk.T + bk
    attn = softmax(q @ k.T, axis=-1)
    out = attn @ x

Math notes:
  * scores = q @ k.T = q @ Wk @ x.T + (q.bk) 1^T. The (q.bk) term is
    constant along the softmax axis, so bk never needs to be computed.
  * q @ Wk = x @ (Wq.T @ Wk) + bq @ Wk: both projections fold into one
    matrix M = Wq.T @ Wk and a row c = bq @ Wk (host-precomputed).
  * Per core (512 query rows):
        tT = M^T . xs^T + c          [D, 512]   (transposed layout)
        S  = t . x^T                 [512, 4096]
        P  = softmax(S)              (online, running-max)
        out = P @ x                  [512, 1024]

Schedule (v3):
  * phase A: M chunks stream on the sync HWDGE ring, xs chunks on the
    scalar ring (independent trigger FIFOs); e-outer over 8 PSUM banks;
    the bias row enters via a K=1 (c x ones) matmul per bank. The
    phase-B chunk-0 stream DMA is interleaved into the M trigger
    sequence so it lands just before phase A's compute finishes.
  * phase B: x^T streams in 512-column chunks (double-buffered).
    Softmax is ONLINE: per (i, chunk) a running negated max is
    maintained on DVE straight out of PSUM, and ACT applies
    exp(PSUM - runmax) directly into bf16 E with accum_out collecting
    per-chunk partial sums. No S buffer exists.
  * after the last chunk per i: c_k = exp(m_k - m_final), Z = sum_k
    z_k c_k, g_k = c_k / Z. The g_k become 32 per-(i,chunk) DIAGONAL
    matrices (bf16), built i-grouped alternating DVE/ACT; phase C
    "transposes" are plain matmuls E_tile @ diag(g) so the softmax
    normalization and running-max corrections ride the mandatory
    transpose for free.
  * phase C: x (bf16) is fully resident in SBUF (8MB, loaded on the
    sync ring behind the phase-B stream; its buffer aliases only the
    phase-A weight pool so the triggers fire as soon as the ring
    drains). Passes over i-tiles {0,1}/{2}/{3}: pass 0 needs only the
    first two diags so its transposes start while phase B's last
    matmuls still run; the last pass leaves just 512KB of output for
    the tail. A single shared PSUM transpose pool avoids cross-pass
    bank WARs (pass 1 lands on never-used banks). Output copy/DMA
    pairs alternate DVE/ACT engines and sync/scalar DMA rings.
"""

import os
from contextlib import ExitStack

import numpy as np
import ml_dtypes

import concourse.bass as bass
import concourse.mybir as mybir
import concourse.tile as tile
from concourse import bacc
from concourse.bass_utils import run_bass_kernel_spmd
from concourse.masks import make_identity

N, D = 4096, 1024
NCORES = 8
R = N // NCORES  # 512 query rows per core
PT = 128  # partition tile
EC = D // PT  # 8 contraction chunks of the model dim
IT = R // PT  # 4 query tiles per core
JC = N // 512  # 8 key chunks of 512
JT = N // PT  # 32 key tiles of 128

F32 = mybir.dt.float32
F32R = mybir.dt.float32r
BF16 = mybir.dt.bfloat16
AX = mybir.AxisListType
AF = mybir.ActivationFunctionType
ALU = mybir.AluOpType

PASSES = ((0, 1), (2,), (3,))


def _emit(nc: bass.Bass, tc: tile.TileContext, aps: dict):
    xTb, xTs, mw, cw, ones, xb, out = (
        aps["xTb"], aps["xTs"], aps["mw"], aps["cw"],
        aps["ones"], aps["xb"], aps["out"],
    )

    with ExitStack() as big:
        persist = big.enter_context(tc.tile_pool(name="persist", bufs=1))

        ident = persist.tile([PT, PT], BF16)
        make_identity(nc, ident)
        c_sb = persist.tile([1, D], F32R)
        ones_sb = persist.tile([1, R], F32R)

        tT_sb = persist.tile([PT, EC, R], F32R)
        E_bf = [persist.tile([PT, N], BF16, name=f"E{i}") for i in range(IT)]
        nmk = [persist.tile([PT, JC], F32, name=f"nmk{i}") for i in range(IT)]
        tmx = [persist.tile([PT, JC], F32, name=f"tmx{i}") for i in range(IT)]
        zpart = [persist.tile([PT, JC], F32, name=f"zp{i}") for i in range(IT)]
        ck = [persist.tile([PT, JC], F32, name=f"ck{i}") for i in range(IT)]
        gk = [persist.tile([PT, JC], F32, name=f"gk{i}") for i in range(IT)]
        zsum = [persist.tile([PT, 1], F32, name=f"z{i}") for i in range(IT)]
        rz = [persist.tile([PT, 1], F32, name=f"rz{i}") for i in range(IT)]
        diag = persist.tile([PT, IT, JC, PT], BF16)

        # opened before wpool so its addresses never overlap the weights;
        # the early stream triggers can then issue during phase A.
        xtpool = big.enter_context(tc.tile_pool(name="xtpool", bufs=3))
        xtjs = {}
        for j in range(JC):
            xtjs[j] = xtpool.tile([PT, EC, 512], F32R, tag="xtj", name="xtj")

        # ---- Phase A: tT = M^T.xs^T + c  (transposed layout)
        with ExitStack() as pha:
            wpool = pha.enter_context(tc.tile_pool(name="wpool", bufs=1))
            apsum = pha.enter_context(tc.tile_pool(name="apsum", bufs=1, space="PSUM"))

            m_sb = wpool.tile([PT, EC, D], F32R)
            xts_sb = wpool.tile([PT, EC, R], F32R)

            m_r = mw.rearrange("(t p) d -> p t d", p=PT)
            xTs_r = xTs.rearrange("(t p) i -> p t i", p=PT)
            # M rides the sync HWDGE ring, xs + bias the scalar ring: the
            # trigger FIFOs are independent and the SDMA engines round-robin
            # between them. Chunk 0 of the phase-B stream is slotted in
            # before the last two M chunks: phase A's compute tail covers it.
            nc.sync.dma_start(m_sb[:, 0, 0:256], m_r[:, 0, 0:256])
            nc.scalar.dma_start(xts_sb[:, 0, :], xTs_r[:, 0, :])
            nc.sync.dma_start(m_sb[:, 0, 256:D], m_r[:, 0, 256:D])
            nc.scalar.dma_start(xts_sb[:, 1, :], xTs_r[:, 1, :])
            nc.scalar.dma_start(c_sb, cw)
            nc.scalar.dma_start(ones_sb, ones)
            for e in range(1, EC - 2):
                nc.sync.dma_start(m_sb[:, e, :], m_r[:, e, :])
            nc.sync.dma_start(xtjs[0], xTb[0])
            for e in range(EC - 2, EC):
                nc.sync.dma_start(m_sb[:, e, :], m_r[:, e, :])
            for e in range(2, EC):
                nc.scalar.dma_start(xts_sb[:, e, :], xTs_r[:, e, :])

            tps = [
                apsum.tile([PT, R], F32, tag=f"tp{d}", name=f"tp{d}")
                for d in range(EC)
            ]
            for e in range(EC):
                for d in range(EC):
                    nc.tensor.matmul(
                        tps[d],
                        m_sb[:, e, d * PT : (d + 1) * PT],
                        xts_sb[:, e, :],
                        start=(e == 0),
                        stop=False,
                    )
            for d in range(EC):
                # bias row: tT[d_block, :] += c[d_block] (x) ones
                nc.tensor.matmul(
                    tps[d],
                    c_sb[:, d * PT : (d + 1) * PT],
                    ones_sb,
                    start=False,
                    stop=True,
                )
                if d % 2 == 0:
                    nc.vector.tensor_copy(tT_sb[:, d, :], tps[d])
                else:
                    nc.scalar.activation(tT_sb[:, d, :], tps[d], func=AF.Copy)

        # ---- Phase B: S chunks in PSUM + online softmax straight to E.
        with ExitStack() as phb:
            spsum = phb.enter_context(tc.tile_pool(name="spsum", bufs=5, space="PSUM"))
            for j in range(JC):
                xtj = xtjs[j]
                if j > 0:
                    nc.sync.dma_start(xtj, xTb[j])
                for i in range(IT):
                    ps = spsum.tile([PT, 512], F32, tag="Sp", name="Sp")
                    for d in range(EC):
                        nc.tensor.matmul(
                            ps,
                            tT_sb[:, d, i * PT : (i + 1) * PT],
                            xtj[:, d, :],
                            start=(d == 0),
                            stop=(d == EC - 1),
                        )
                    if j == 0:
                        nc.vector.reduce_max(
                            out=nmk[i][:, 0:1], in_=ps, axis=AX.X, negate=True
                        )
                    else:
                        nc.vector.reduce_max(
                            out=tmx[i][:, j : j + 1], in_=ps, axis=AX.X, negate=True
                        )
                        nc.vector.tensor_tensor(
                            out=nmk[i][:, j : j + 1],
                            in0=nmk[i][:, j - 1 : j],
                            in1=tmx[i][:, j : j + 1],
                            op=ALU.min,
                        )
                    nc.scalar.activation(
                        out=E_bf[i][:, j * 512 : (j + 1) * 512],
                        in_=ps,
                        func=AF.Exp,
                        bias=nmk[i][:, j : j + 1],
                        scale=1.0,
                        accum_out=zpart[i][:, j : j + 1],
                    )

            # finalize: c_k = exp(m_k - m_last), Z = sum z_k c_k, g = c_k/Z;
            # then the 8 per-chunk diag(g) tiles, i-grouped so pass 0 of
            # phase C (i-tiles 0,1) unblocks while i2/i3 still matmul.
            for i in range(IT):
                nc.scalar.activation(
                    out=ck[i],
                    in_=nmk[i],
                    func=AF.Exp,
                    bias=nmk[i][:, JC - 1 : JC],
                    scale=-1.0,
                )
                nc.vector.tensor_tensor(
                    out=gk[i], in0=zpart[i], in1=ck[i], op=ALU.mult
                )
                nc.vector.reduce_sum(out=zsum[i], in_=gk[i], axis=AX.X)
                nc.vector.reciprocal(rz[i], zsum[i])
                nc.vector.tensor_scalar_mul(gk[i], ck[i], rz[i])
                for k in range(JC):
                    if k % 2 == 0:
                        nc.vector.tensor_scalar_mul(
                            diag[:, i, k, :], ident, gk[i][:, k : k + 1]
                        )
                    else:
                        nc.scalar.activation(
                            diag[:, i, k, :],
                            ident,
                            func=AF.Copy,
                            scale=gk[i][:, k : k + 1],
                        )

        # ---- Phase C: out = P @ x with x fully resident in SBUF.
        # xb reuses the phase-A weight pool's address range; its triggers sit
        # on the sync ring behind the phase-B stream.
        xbpool = big.enter_context(tc.tile_pool(name="xbpool", bufs=1))
        xb_sb = xbpool.tile([PT, JT, D], BF16)
        xb_r = xb.rearrange("(t p) d -> p t d", p=PT)
        for g in range(8):
            nc.sync.dma_start(
                xb_sb[:, 4 * g : 4 * g + 4, :], xb_r[:, 4 * g : 4 * g + 4, :]
            )
        etpool = big.enter_context(tc.tile_pool(name="etpool", bufs=4))
        ocopy = big.enter_context(tc.tile_pool(name="ocopy", bufs=3))
        tpsum = big.enter_context(tc.tile_pool(name="tpsum", bufs=2, space="PSUM"))

        LOOK = 2
        for pi, ii in enumerate(PASSES):
            W = len(ii) * PT
            with ExitStack() as phc:
                opsum = phc.enter_context(
                    tc.tile_pool(name=f"opsum{pi}", bufs=1, space="PSUM")
                )
                oacc = {
                    (i, dn): opsum.tile(
                        [PT, 512], F32, tag=f"o{i}_{dn}", name=f"o{i}_{dn}"
                    )
                    for i in ii
                    for dn in range(2)
                }
                ets = {}
                for jtv in range(JT + LOOK):
                    if jtv < JT:
                        jt = jtv
                        # "transpose" = E_tile.T @ diag(g): per-row softmax
                        # scale applied for free by the mandatory transpose.
                        pst = tpsum.tile([PT, 2 * PT], F32, tag="tp", name="pst")
                        for kp, i in enumerate(ii):
                            nc.tensor.matmul(
                                pst[:, kp * PT : (kp + 1) * PT],
                                E_bf[i][:, jt * PT : (jt + 1) * PT],
                                diag[:, i, jt // 4, :],
                                start=True,
                                stop=True,
                                skip_group_check=True,
                            )
                        et = etpool.tile([PT, W], BF16, tag=f"et{pi}", name="et")
                        if jt % 2 == 0:
                            nc.vector.tensor_copy(et, pst[:, 0:W])
                        else:
                            nc.scalar.activation(et, pst[:, 0:W], func=AF.Copy)
                        ets[jt % 4] = et
                    if jtv >= LOOK:
                        jt = jtv - LOOK
                        for kp, i in enumerate(ii):
                            for dn in range(2):
                                nc.tensor.matmul(
                                    oacc[(i, dn)],
                                    ets[jt % 4][:, kp * PT : (kp + 1) * PT],
                                    xb_sb[:, jt, dn * 512 : (dn + 1) * 512],
                                    start=(jt == 0),
                                    stop=(jt == JT - 1),
                                )
                for kp, i in enumerate(ii):
                    for dn in range(2):
                        ot = ocopy.tile([PT, 512], F32, tag="ot", name="ot")
                        if dn == 0:
                            nc.vector.tensor_copy(ot, oacc[(i, dn)])
                            nc.sync.dma_start(
                                out[i * PT : (i + 1) * PT, 0:512], ot
                            )
                        else:
                            nc.scalar.activation(ot, oacc[(i, dn)], func=AF.Copy)
                            nc.scalar.dma_start(
                                out[i * PT : (i + 1) * PT, 512:1024], ot
                            )


def build():
    nc = bacc.Bacc(
        "TRN2",
        target_bir_lowering=False,
        debug=False,
        enable_asserts=False,
        num_devices=NCORES,
    )
    aps = {
        "xTb": nc.dram_tensor("xTb", [JC, PT, EC, 512], F32R, kind="ExternalInput").ap(),
        "xTs": nc.dram_tensor("xTs", [D, R], F32R, kind="ExternalInput").ap(),
        "mw": nc.dram_tensor("mw", [D, D], F32R, kind="ExternalInput").ap(),
        "cw": nc.dram_tensor("cw", [1, D], F32R, kind="ExternalInput").ap(),
        "ones": nc.dram_tensor("ones", [1, R], F32R, kind="ExternalInput").ap(),
        "xb": nc.dram_tensor("xb", [N, D], BF16, kind="ExternalInput").ap(),
        "out": nc.dram_tensor("out", [R, D], F32, kind="ExternalOutput").ap(),
    }
    with tile.TileContext(nc) as tc:
        _emit(nc, tc, aps)
    nc.compile()
    return nc


_NC_CACHE = None
LAST_RESULTS = None


def _get_nc():
    global _NC_CACHE
    if _NC_CACHE is None:
        _NC_CACHE = build()
    return _NC_CACHE


def make_in_maps(x, Wq, bq, Wk):
    x = np.ascontiguousarray(np.asarray(x, dtype=np.float32))
    xT = np.ascontiguousarray(x.T)
    # xTb[j, p, e, n] = xT[e*128 + p, j*512 + n]: per-(j,p) contiguous 16KB
    # blocks so the phase-B stream DMAs at full descriptor size.
    xTb = np.ascontiguousarray(
        xT.reshape(EC, PT, JC, 512).transpose(2, 1, 0, 3)
    )
    wk64 = np.asarray(Wk, dtype=np.float64)
    mw = np.ascontiguousarray(
        (np.asarray(Wq, dtype=np.float64).T @ wk64).astype(np.float32)
    )
    cw = np.ascontiguousarray(
        (np.asarray(bq, dtype=np.float64) @ wk64).astype(np.float32).reshape(1, D)
    )
    ones_arr = np.ones((1, R), dtype=np.float32)
    xb = x.astype(ml_dtypes.bfloat16)
    in_maps = []
    for c in range(NCORES):
        in_maps.append(
            {
                "xTb": xTb,
                "xTs": np.ascontiguousarray(xT[:, c * R : (c + 1) * R]),
                "mw": mw,
                "cw": cw,
                "ones": ones_arr,
                "xb": xb,
            }
        )
    return in_maps


def kernel(x, Wq, bq, Wk, bk):
    # bk only shifts each score row by a constant, which softmax cancels.
    del bk
    in_maps = make_in_maps(x, Wq, bq, Wk)
    nc = _get_nc()
    kwargs = {}
    if os.environ.get("K_TRACE_DIR"):
        import tempfile

        kwargs["tmpdir"] = tempfile.mkdtemp(dir=os.environ["K_TRACE_DIR"])
    res = run_bass_kernel_spmd(nc, in_maps, core_ids=list(range(NCORES)), **kwargs)
    global LAST_RESULTS
    LAST_RESULTS = res
    return np.concatenate(
        [np.asarray(res.results[c]["out"], dtype=np.float32) for c in range(NCORES)],
        axis=0,
    )


# revision 8
# speedup vs baseline: 1.1477x; 1.0562x over previous
"""Trainium2 Bass kernel for CLIP attention pooling.

Reference computation (N=4096, D=1024, fp32):
    q = x @ Wq.T + bq
    k = x @ Wk.T + bk
    attn = softmax(q @ k.T, axis=-1)
    out = attn @ x

Math notes:
  * scores = q @ k.T = q @ Wk @ x.T + (q.bk) 1^T. The (q.bk) term is
    constant along the softmax axis, so bk never needs to be computed.
  * q @ Wk = x @ (Wq.T @ Wk) + bq @ Wk: both projections fold into one
    matrix M = Wq.T @ Wk and a row c = bq @ Wk (host-precomputed).
  * Per core (512 query rows):
        tT = M^T . xs^T + c          [D, 512]   (transposed layout)
        S  = t . x^T                 [512, 4096]
        P  = softmax(S)              (online, running-max)
        out = P @ x                  [512, 1024]

Schedule (v3):
  * phase A: M chunks stream on the sync HWDGE ring, xs chunks on the
    scalar ring (independent trigger FIFOs); e-outer over 8 PSUM banks;
    the bias row enters via a K=1 (c x ones) matmul per bank. The
    phase-B chunk-0 stream DMA is interleaved into the M trigger
    sequence so it lands just before phase A's compute finishes.
  * phase B: x^T streams in 512-column chunks (double-buffered).
    Softmax is ONLINE: per (i, chunk) a running negated max is
    maintained on DVE straight out of PSUM, and ACT applies
    exp(PSUM - runmax) directly into bf16 E with accum_out collecting
    per-chunk partial sums. No S buffer exists.
  * after the last chunk per i: c_k = exp(m_k - m_final), Z = sum_k
    z_k c_k, g_k = c_k / Z. The g_k become 32 per-(i,chunk) DIAGONAL
    matrices (bf16), built i-grouped alternating DVE/ACT; phase C
    "transposes" are plain matmuls E_tile @ diag(g) so the softmax
    normalization and running-max corrections ride the mandatory
    transpose for free.
  * phase C: x (bf16) is fully resident in SBUF (8MB, loaded on the
    sync ring behind the phase-B stream; its buffer aliases only the
    phase-A weight pool so the triggers fire as soon as the ring
    drains). Passes over i-tiles {0,1}/{2}/{3}: pass 0 needs only the
    first two diags so its transposes start while phase B's last
    matmuls still run; the last pass leaves just 512KB of output for
    the tail. A single shared PSUM transpose pool avoids cross-pass
    bank WARs (pass 1 lands on never-used banks). Output copy/DMA
    pairs alternate DVE/ACT engines and sync/scalar DMA rings.
"""

import os
from contextlib import ExitStack

import numpy as np
import ml_dtypes

import concourse.bass as bass
import concourse.mybir as mybir
import concourse.tile as tile
from concourse import bacc
from concourse.bass_utils import run_bass_kernel_spmd
from concourse.masks import make_identity

N, D = 4096, 1024
NCORES = 8
R = N // NCORES  # 512 query rows per core
PT = 128  # partition tile
EC = D // PT  # 8 contraction chunks of the model dim
IT = R // PT  # 4 query tiles per core
JC = N // 512  # 8 key chunks of 512
JT = N // PT  # 32 key tiles of 128

F32 = mybir.dt.float32
F32R = mybir.dt.float32r
BF16 = mybir.dt.bfloat16
AX = mybir.AxisListType
AF = mybir.ActivationFunctionType
ALU = mybir.AluOpType

PASSES = ((0, 1), (2,), (3,))


def _emit(nc: bass.Bass, tc: tile.TileContext, aps: dict):
    xTb, xTs, mw, cw, ones, xb, out = (
        aps["xTb"], aps["xTs"], aps["mw"], aps["cw"],
        aps["ones"], aps["xb"], aps["out"],
    )

    with ExitStack() as big:
        persist = big.enter_context(tc.tile_pool(name="persist", bufs=1))

        ident = persist.tile([PT, PT], BF16)
        make_identity(nc, ident)
        c_sb = persist.tile([1, D], F32R)
        ones_sb = persist.tile([1, R], F32R)

        tT_sb = persist.tile([PT, EC, R], F32R)
        E_bf = [persist.tile([PT, N], BF16, name=f"E{i}") for i in range(IT)]
        nmk = [persist.tile([PT, JC], F32, name=f"nmk{i}") for i in range(IT)]
        tmx = [persist.tile([PT, JC], F32, name=f"tmx{i}") for i in range(IT)]
        zpart = [persist.tile([PT, JC], F32, name=f"zp{i}") for i in range(IT)]
        ck = [persist.tile([PT, JC], F32, name=f"ck{i}") for i in range(IT)]
        gk = [persist.tile([PT, JC], F32, name=f"gk{i}") for i in range(IT)]
        zsum = [persist.tile([PT, 1], F32, name=f"z{i}") for i in range(IT)]
        rz = [persist.tile([PT, 1], F32, name=f"rz{i}") for i in range(IT)]
        diag = persist.tile([PT, IT, JC, PT], BF16)

        # opened before wpool so its addresses never overlap the weights;
        # the early stream triggers can then issue during phase A.
        xtpool = big.enter_context(tc.tile_pool(name="xtpool", bufs=3))
        xtjs = {}
        for j in range(JC):
            xtjs[j] = xtpool.tile([PT, EC, 512], F32R, tag="xtj", name="xtj")

        # ---- Phase A: tT = M^T.xs^T + c  (transposed layout)
        with ExitStack() as pha:
            wpool = pha.enter_context(tc.tile_pool(name="wpool", bufs=1))
            apsum = pha.enter_context(tc.tile_pool(name="apsum", bufs=1, space="PSUM"))

            m_sb = wpool.tile([PT, EC, D], F32R)
            xts_sb = wpool.tile([PT, EC, R], F32R)

            m_r = mw.rearrange("(t p) d -> p t d", p=PT)
            xTs_r = xTs.rearrange("(t p) i -> p t i", p=PT)
            # M rides the sync HWDGE ring, xs + bias the scalar ring: the
            # trigger FIFOs are independent and the SDMA engines round-robin
            # between them. Chunk 0 of the phase-B stream is slotted in
            # before the last two M chunks: phase A's compute tail covers it.
            nc.sync.dma_start(m_sb[:, 0, 0:256], m_r[:, 0, 0:256])
            nc.scalar.dma_start(xts_sb[:, 0, :], xTs_r[:, 0, :])
            nc.sync.dma_start(m_sb[:, 0, 256:D], m_r[:, 0, 256:D])
            nc.scalar.dma_start(xts_sb[:, 1, :], xTs_r[:, 1, :])
            nc.scalar.dma_start(c_sb, cw)
            nc.scalar.dma_start(ones_sb, ones)
            for e in range(1, EC):
                nc.sync.dma_start(m_sb[:, e, :], m_r[:, e, :])
            nc.sync.dma_start(xtjs[0], xTb[0])
            for e in range(2, EC):
                nc.scalar.dma_start(xts_sb[:, e, :], xTs_r[:, e, :])

            tps = [
                apsum.tile([PT, R], F32, tag=f"tp{d}", name=f"tp{d}")
                for d in range(EC)
            ]
            for e in range(EC):
                for d in range(EC):
                    nc.tensor.matmul(
                        tps[d],
                        m_sb[:, e, d * PT : (d + 1) * PT],
                        xts_sb[:, e, :],
                        start=(e == 0),
                        stop=False,
                    )
            for d in range(EC):
                # bias row: tT[d_block, :] += c[d_block] (x) ones
                nc.tensor.matmul(
                    tps[d],
                    c_sb[:, d * PT : (d + 1) * PT],
                    ones_sb,
                    start=False,
                    stop=True,
                )
                if d % 2 == 0:
                    nc.vector.tensor_copy(tT_sb[:, d, :], tps[d])
                else:
                    nc.scalar.activation(tT_sb[:, d, :], tps[d], func=AF.Copy)

        # ---- Phase B: S chunks in PSUM + online softmax straight to E.
        def softmax_step(ps, i, j):
            if j == 0:
                nc.vector.reduce_max(
                    out=nmk[i][:, 0:1], in_=ps, axis=AX.X, negate=True
                )
            else:
                nc.vector.reduce_max(
                    out=tmx[i][:, j : j + 1], in_=ps, axis=AX.X, negate=True
                )
                nc.vector.tensor_tensor(
                    out=nmk[i][:, j : j + 1],
                    in0=nmk[i][:, j - 1 : j],
                    in1=tmx[i][:, j : j + 1],
                    op=ALU.min,
                )
            nc.scalar.activation(
                out=E_bf[i][:, j * 512 : (j + 1) * 512],
                in_=ps,
                func=AF.Exp,
                bias=nmk[i][:, j : j + 1],
                scale=1.0,
                accum_out=zpart[i][:, j : j + 1],
            )

        def finalize_pair(ia, ib):
            # c_k = exp(m_k - m_last), Z = sum z_k c_k, g = c_k/Z; then the
            # per-chunk diag(g) tiles, k-ordered round-robin across DVE/ACT
            # so both i-tiles' early-k diags finish first, in parallel.
            for i in (ia, ib):
                nc.scalar.activation(
                    out=ck[i],
                    in_=nmk[i],
                    func=AF.Exp,
                    bias=nmk[i][:, JC - 1 : JC],
                    scale=-1.0,
                )
            for i in (ia, ib):
                nc.vector.tensor_tensor(
                    out=gk[i], in0=zpart[i], in1=ck[i], op=ALU.mult
                )
            for i in (ia, ib):
                nc.vector.reduce_sum(out=zsum[i], in_=gk[i], axis=AX.X)
            for i in (ia, ib):
                nc.vector.reciprocal(rz[i], zsum[i])
            for i in (ia, ib):
                nc.vector.tensor_scalar_mul(gk[i], ck[i], rz[i])
            for k in range(JC):
                dve_i = ia if k % 2 == 0 else ib
                act_i = ib if k % 2 == 0 else ia
                nc.vector.tensor_scalar_mul(
                    diag[:, dve_i, k, :], ident, gk[dve_i][:, k : k + 1]
                )
                nc.scalar.activation(
                    diag[:, act_i, k, :],
                    ident,
                    func=AF.Copy,
                    scale=gk[act_i][:, k : k + 1],
                )

        with ExitStack() as phb:
            spsum = phb.enter_context(tc.tile_pool(name="spsum", bufs=5, space="PSUM"))
            for j in range(JC):
                xtj = xtjs[j]
                if j > 0:
                    nc.sync.dma_start(xtj, xTb[j])
                last = j == JC - 1
                pss = []
                for i in range(IT):
                    ps = spsum.tile([PT, 512], F32, tag="Sp", name="Sp")
                    pss.append(ps)
                    for d in range(EC):
                        nc.tensor.matmul(
                            ps,
                            tT_sb[:, d, i * PT : (i + 1) * PT],
                            xtj[:, d, :],
                            start=(d == 0),
                            stop=(d == EC - 1),
                        )
                    if not last:
                        softmax_step(ps, i, j)
                if last:
                    # emit i0/i1's softmax + finalize ahead of i2/i3's, so
                    # phase C's first pass unblocks right at B's compute end.
                    for i in (0, 1):
                        softmax_step(pss[i], i, j)
                    finalize_pair(0, 1)
                    for i in (2, 3):
                        softmax_step(pss[i], i, j)
                    finalize_pair(2, 3)

        # ---- Phase C: out = P @ x with x fully resident in SBUF.
        # xb reuses the phase-A weight pool's address range; its triggers sit
        # on the sync ring behind the phase-B stream.
        xbpool = big.enter_context(tc.tile_pool(name="xbpool", bufs=1))
        xb_sb = xbpool.tile([PT, JT, D], BF16)
        xb_r = xb.rearrange("(t p) d -> p t d", p=PT)
        for g in range(8):
            nc.sync.dma_start(
                xb_sb[:, 4 * g : 4 * g + 4, :], xb_r[:, 4 * g : 4 * g + 4, :]
            )
        etpool = big.enter_context(tc.tile_pool(name="etpool", bufs=3))
        ocopy = big.enter_context(tc.tile_pool(name="ocopy", bufs=4))
        # opsum allocated before tpsum: oacc lands on the banks whose phase-B
        # WARs clear first (their mms only start LOOKP pairs in), tpsum gets
        # the late-released + fresh banks so the first transposes don't wait
        # on i2/i3's last exps.
        opsum = big.enter_context(tc.tile_pool(name="opsum", bufs=1, space="PSUM"))
        tpsum = big.enter_context(tc.tile_pool(name="tpsum", bufs=3, space="PSUM"))
        # accumulators are shared by both passes (keyed by position kp);
        # pass 1's first start=True matmuls depend on pass 0's drain copies,
        # which are emitted interleaved into pass 1's first pairs.
        oacc = {
            (kp, dn): opsum.tile([PT, 512], F32, tag=f"o{kp}_{dn}", name=f"o{kp}_{dn}")
            for kp in range(2)
            for dn in range(2)
        }

        def drain_item(kp, dn, i):
            def emit():
                ot = ocopy.tile([PT, 512], F32, tag="ot", name="ot")
                if dn == 0:
                    nc.vector.tensor_copy(ot, oacc[(kp, dn)])
                    nc.sync.dma_start(out[i * PT : (i + 1) * PT, 0:512], ot)
                else:
                    nc.scalar.activation(ot, oacc[(kp, dn)], func=AF.Copy)
                    nc.scalar.dma_start(out[i * PT : (i + 1) * PT, 512:1024], ot)

            return emit

        NP = JT // 2  # jt pairs
        LOOKP = 2
        pending = []
        for pi, ii in enumerate(((0, 1), (2, 3))):
            ets = {}
            for pv in range(NP + LOOKP):
                if pv < NP:
                    # "transpose" = E_tile.T @ diag(g): per-row softmax scale
                    # applied for free by the mandatory transpose. Two jt per
                    # pst bank -> one PSUM->SBUF copy per pair.
                    pst = tpsum.tile([PT, 512], F32, tag="tp", name="pst")
                    for kp, i in enumerate(ii):
                        for s in range(2):
                            jt = 2 * pv + s
                            nc.tensor.matmul(
                                pst[:, (2 * kp + s) * PT : (2 * kp + s + 1) * PT],
                                E_bf[i][:, jt * PT : (jt + 1) * PT],
                                diag[:, i, jt // 4, :],
                                start=True,
                                stop=True,
                                skip_group_check=True,
                            )
                    et = etpool.tile([PT, 512], BF16, tag="et", name="et")
                    if pv % 2 == 0:
                        nc.vector.tensor_copy(et, pst)
                    else:
                        nc.scalar.activation(et, pst, func=AF.Copy)
                    ets[pv % 3] = et
                    # two per pair: all of the previous pass's drains must be
                    # emitted before this pass's first start=True matmuls
                    # (pv == LOOKP) reuse the accumulator banks.
                    for _ in range(2):
                        if pending:
                            pending.pop(0)()
                if pv >= LOOKP:
                    p = pv - LOOKP
                    for s in range(2):
                        jt = 2 * p + s
                        for kp, i in enumerate(ii):
                            for dn in range(2):
                                nc.tensor.matmul(
                                    oacc[(kp, dn)],
                                    ets[p % 3][
                                        :, (2 * kp + s) * PT : (2 * kp + s + 1) * PT
                                    ],
                                    xb_sb[:, jt, dn * 512 : (dn + 1) * 512],
                                    start=(jt == 0),
                                    stop=(jt == JT - 1),
                                )
            drains = [
                drain_item(kp, dn, i)
                for kp, i in enumerate(ii)
                for dn in range(2)
            ]
            if pi == 0:
                pending = drains
            else:
                for d in drains:
                    d()


def build():
    nc = bacc.Bacc(
        "TRN2",
        target_bir_lowering=False,
        debug=False,
        enable_asserts=False,
        num_devices=NCORES,
    )
    aps = {
        "xTb": nc.dram_tensor("xTb", [JC, PT, EC, 512], F32R, kind="ExternalInput").ap(),
        "xTs": nc.dram_tensor("xTs", [D, R], F32R, kind="ExternalInput").ap(),
        "mw": nc.dram_tensor("mw", [D, D], F32R, kind="ExternalInput").ap(),
        "cw": nc.dram_tensor("cw", [1, D], F32R, kind="ExternalInput").ap(),
        "ones": nc.dram_tensor("ones", [1, R], F32R, kind="ExternalInput").ap(),
        "xb": nc.dram_tensor("xb", [N, D], BF16, kind="ExternalInput").ap(),
        "out": nc.dram_tensor("out", [R, D], F32, kind="ExternalOutput").ap(),
    }
    with tile.TileContext(nc) as tc:
        _emit(nc, tc, aps)
    nc.compile()
    return nc


_NC_CACHE = None
LAST_RESULTS = None


def _get_nc():
    global _NC_CACHE
    if _NC_CACHE is None:
        _NC_CACHE = build()
    return _NC_CACHE


def make_in_maps(x, Wq, bq, Wk):
    x = np.ascontiguousarray(np.asarray(x, dtype=np.float32))
    xT = np.ascontiguousarray(x.T)
    # xTb[j, p, e, n] = xT[e*128 + p, j*512 + n]: per-(j,p) contiguous 16KB
    # blocks so the phase-B stream DMAs at full descriptor size.
    xTb = np.ascontiguousarray(
        xT.reshape(EC, PT, JC, 512).transpose(2, 1, 0, 3)
    )
    wk64 = np.asarray(Wk, dtype=np.float64)
    mw = np.ascontiguousarray(
        (np.asarray(Wq, dtype=np.float64).T @ wk64).astype(np.float32)
    )
    cw = np.ascontiguousarray(
        (np.asarray(bq, dtype=np.float64) @ wk64).astype(np.float32).reshape(1, D)
    )
    ones_arr = np.ones((1, R), dtype=np.float32)
    xb = x.astype(ml_dtypes.bfloat16)
    in_maps = []
    for c in range(NCORES):
        in_maps.append(
            {
                "xTb": xTb,
                "xTs": np.ascontiguousarray(xT[:, c * R : (c + 1) * R]),
                "mw": mw,
                "cw": cw,
                "ones": ones_arr,
                "xb": xb,
            }
        )
    return in_maps


def kernel(x, Wq, bq, Wk, bk):
    # bk only shifts each score row by a constant, which softmax cancels.
    del bk
    in_maps = make_in_maps(x, Wq, bq, Wk)
    nc = _get_nc()
    kwargs = {}
    if os.environ.get("K_TRACE_DIR"):
        import tempfile

        kwargs["tmpdir"] = tempfile.mkdtemp(dir=os.environ["K_TRACE_DIR"])
    res = run_bass_kernel_spmd(nc, in_maps, core_ids=list(range(NCORES)), **kwargs)
    global LAST_RESULTS
    LAST_RESULTS = res
    return np.concatenate(
        [np.asarray(res.results[c]["out"], dtype=np.float32) for c in range(NCORES)],
        axis=0,
    )


# revision 16
# speedup vs baseline: 1.1863x; 1.0336x over previous
"""Trainium2 Bass kernel for CLIP attention pooling.

Reference computation (N=4096, D=1024, fp32):
    q = x @ Wq.T + bq
    k = x @ Wk.T + bk
    attn = softmax(q @ k.T, axis=-1)
    out = attn @ x

Math notes:
  * scores = q @ k.T = q @ Wk @ x.T + (q.bk) 1^T. The (q.bk) term is
    constant along the softmax axis, so bk never needs to be computed.
  * q @ Wk = x @ (Wq.T @ Wk) + bq @ Wk: both projections fold into one
    matrix M = Wq.T @ Wk and a row c = bq @ Wk (host-precomputed).
  * Per core (512 query rows):
        tT = M^T . xs^T + c          [D, 512]   (transposed layout)
        S  = t . x^T                 [512, 4096]
        P  = softmax(S)              (online, running-max)
        out = P @ x                  [512, 1024]

Schedule (v3):
  * phase A: M chunks stream on the sync HWDGE ring, xs chunks on the
    scalar ring (independent trigger FIFOs); e-outer over 8 PSUM banks;
    the bias row enters via a K=1 (c x ones) matmul per bank. The
    phase-B chunk-0 stream DMA is interleaved into the M trigger
    sequence so it lands just before phase A's compute finishes.
  * phase B: x^T streams in 512-column chunks (double-buffered).
    Softmax is ONLINE: per (i, chunk) a running negated max is
    maintained on DVE straight out of PSUM, and ACT applies
    exp(PSUM - runmax) directly into bf16 E with accum_out collecting
    per-chunk partial sums. No S buffer exists.
  * after the last chunk per i: c_k = exp(m_k - m_final), Z = sum_k
    z_k c_k, g_k = c_k / Z. The g_k become 32 per-(i,chunk) DIAGONAL
    matrices (bf16), built i-grouped alternating DVE/ACT; phase C
    "transposes" are plain matmuls E_tile @ diag(g) so the softmax
    normalization and running-max corrections ride the mandatory
    transpose for free.
  * phase C: x (bf16) is fully resident in SBUF (8MB, loaded on the
    sync ring behind the phase-B stream; its buffer aliases only the
    phase-A weight pool so the triggers fire as soon as the ring
    drains). Passes over i-tiles {0,1}/{2}/{3}: pass 0 needs only the
    first two diags so its transposes start while phase B's last
    matmuls still run; the last pass leaves just 512KB of output for
    the tail. A single shared PSUM transpose pool avoids cross-pass
    bank WARs (pass 1 lands on never-used banks). Output copy/DMA
    pairs alternate DVE/ACT engines and sync/scalar DMA rings.
"""

import os
from contextlib import ExitStack

import numpy as np
import ml_dtypes

import concourse.bass as bass
import concourse.mybir as mybir
import concourse.tile as tile
from concourse import bacc
from concourse.bass_utils import run_bass_kernel_spmd
from concourse.masks import make_identity

N, D = 4096, 1024
NCORES = 8
R = N // NCORES  # 512 query rows per core
PT = 128  # partition tile
EC = D // PT  # 8 contraction chunks of the model dim
IT = R // PT  # 4 query tiles per core
JC = N // 512  # 8 key chunks of 512
JT = N // PT  # 32 key tiles of 128

F32 = mybir.dt.float32
F32R = mybir.dt.float32r
BF16 = mybir.dt.bfloat16
AX = mybir.AxisListType
AF = mybir.ActivationFunctionType
ALU = mybir.AluOpType

PASSES = ((0, 1), (2,), (3,))


def _emit(nc: bass.Bass, tc: tile.TileContext, aps: dict):
    xTb, xTs, mw, cw, xb, out = (
        aps["xTb"], aps["xTs"], aps["mw"], aps["cw"],
        aps["xb"], aps["out"],
    )

    with ExitStack() as big:
        persist = big.enter_context(tc.tile_pool(name="persist", bufs=1))

        ident = persist.tile([PT, PT], BF16)
        make_identity(nc, ident)
        c_sb = persist.tile([PT, EC], F32)

        tT_sb = persist.tile([PT, EC, R], F32R)
        E_bf = [persist.tile([PT, N], BF16, name=f"E{i}") for i in range(IT)]
        nmk = [persist.tile([PT, JC], F32, name=f"nmk{i}") for i in range(IT)]
        tmx = [persist.tile([PT, JC], F32, name=f"tmx{i}") for i in range(IT)]
        zpart = [persist.tile([PT, JC], F32, name=f"zp{i}") for i in range(IT)]
        ck = [persist.tile([PT, JC], F32, name=f"ck{i}") for i in range(IT)]
        gk = [persist.tile([PT, JC], F32, name=f"gk{i}") for i in range(IT)]
        zsum = [persist.tile([PT, 1], F32, name=f"z{i}") for i in range(IT)]
        rz = [persist.tile([PT, 1], F32, name=f"rz{i}") for i in range(IT)]
        diag = persist.tile([PT, IT, JC, PT], BF16)

        # opened before wpool so its addresses never overlap the weights;
        # the early stream triggers can then issue during phase A.
        xtpool = big.enter_context(tc.tile_pool(name="xtpool", bufs=3))
        xtjs = {}
        for j in range(1, JC):
            xtjs[j] = xtpool.tile([PT, EC, 512], F32R, tag="xtj", name="xtj")

        # xs in its own pool (opened after xtpool so pools unwind LIFO):
        # stream position 0 of phase B reads it directly - each core's query
        # slice IS its own key chunk - so it is only released after that;
        # the xb buffer then aliases it + wpool.
        xspool_cm = tc.tile_pool(name="xspool", bufs=1)
        xspool = xspool_cm.__enter__()
        xts_sb = xspool.tile([PT, EC, R], F32R)

        # ---- Phase A: tT = M^T.xs^T + c  (transposed layout)
        with ExitStack() as pha:
            wpool = pha.enter_context(tc.tile_pool(name="wpool", bufs=1))
            apsum = pha.enter_context(tc.tile_pool(name="apsum", bufs=1, space="PSUM"))

            m_sb = wpool.tile([PT, EC, D], F32R)

            m_r = mw.rearrange("(t p) d -> p t d", p=PT)
            xTs_r = xTs.rearrange("(t p) i -> p t i", p=PT)
            # M rides the sync HWDGE ring, xs + bias the scalar ring: the
            # trigger FIFOs are independent and the SDMA engines round-robin
            # between them. Chunk 0 of the phase-B stream is slotted in
            # before the last two M chunks: phase A's compute tail covers it.
            nc.sync.dma_start(m_sb[:, 0, 0:256], m_r[:, 0, 0:256])
            nc.scalar.dma_start(xts_sb[:, 0, :], xTs_r[:, 0, :])
            nc.sync.dma_start(m_sb[:, 0, 256:D], m_r[:, 0, 256:D])
            nc.scalar.dma_start(xts_sb[:, 1, :], xTs_r[:, 1, :])
            nc.scalar.dma_start(c_sb, cw)
            for e in range(1, EC):
                nc.sync.dma_start(m_sb[:, e, :], m_r[:, e, :])
            for e in range(2, EC):
                nc.scalar.dma_start(xts_sb[:, e, :], xTs_r[:, e, :])

            tps = [
                apsum.tile([PT, R], F32, tag=f"tp{d}", name=f"tp{d}")
                for d in range(EC)
            ]
            for e in range(EC):
                for d in range(EC):
                    nc.tensor.matmul(
                        tps[d],
                        m_sb[:, e, d * PT : (d + 1) * PT],
                        xts_sb[:, e, :],
                        start=(e == 0),
                        stop=(e == EC - 1),
                    )
            for d in range(EC):
                # bias folded into the PSUM->SBUF copy: tT[d_blk,:] += c[d_blk]
                if d % 2 == 0:
                    nc.vector.tensor_scalar_add(
                        tT_sb[:, d, :], tps[d], c_sb[:, d : d + 1]
                    )
                else:
                    nc.scalar.activation(
                        tT_sb[:, d, :], tps[d], func=AF.Identity,
                        bias=c_sb[:, d : d + 1],
                    )

        # ---- Phase B: S chunks in PSUM + online softmax straight to E.
        def softmax_step(ps, i, j):
            if j == 0:
                nc.vector.reduce_max(
                    out=nmk[i][:, 0:1], in_=ps, axis=AX.X, negate=True
                )
            else:
                nc.vector.reduce_max(
                    out=tmx[i][:, j : j + 1], in_=ps, axis=AX.X, negate=True
                )
                nc.vector.tensor_tensor(
                    out=nmk[i][:, j : j + 1],
                    in0=nmk[i][:, j - 1 : j],
                    in1=tmx[i][:, j : j + 1],
                    op=ALU.min,
                )
            nc.scalar.activation(
                out=E_bf[i][:, j * 512 : (j + 1) * 512],
                in_=ps,
                func=AF.Exp,
                bias=nmk[i][:, j : j + 1],
                scale=1.0,
                accum_out=zpart[i][:, j : j + 1],
            )

        def finalize_pair(ia, ib):
            # c_k = exp(m_k - m_last), Z = sum z_k c_k, g = c_k/Z; then the
            # per-chunk diag(g) tiles, k-ordered round-robin across DVE/ACT
            # so both i-tiles' early-k diags finish first, in parallel.
            for i in (ia, ib):
                nc.scalar.activation(
                    out=ck[i],
                    in_=nmk[i],
                    func=AF.Exp,
                    bias=nmk[i][:, JC - 1 : JC],
                    scale=-1.0,
                )
            for i in (ia, ib):
                nc.vector.tensor_tensor(
                    out=gk[i], in0=zpart[i], in1=ck[i], op=ALU.mult
                )
            for i in (ia, ib):
                nc.vector.reduce_sum(out=zsum[i], in_=gk[i], axis=AX.X)
            for i in (ia, ib):
                nc.vector.reciprocal(rz[i], zsum[i])
            for i in (ia, ib):
                nc.vector.tensor_scalar_mul(gk[i], ck[i], rz[i])
            for k in range(JC):
                dve_i = ia if k % 2 == 0 else ib
                act_i = ib if k % 2 == 0 else ia
                nc.vector.tensor_scalar_mul(
                    diag[:, dve_i, k, :], ident, gk[dve_i][:, k : k + 1]
                )
                nc.scalar.activation(
                    diag[:, act_i, k, :],
                    ident,
                    func=AF.Copy,
                    scale=gk[act_i][:, k : k + 1],
                )

        bpend = []
        with ExitStack() as phb:
            spsum = phb.enter_context(tc.tile_pool(name="spsum", bufs=4, space="PSUM"))
            padpool = phb.enter_context(
                tc.tile_pool(name="padpool", bufs=1, space="PSUM")
            )
            def mm_group(ps, i, xtj):
                for d in range(EC):
                    nc.tensor.matmul(
                        ps,
                        tT_sb[:, d, i * PT : (i + 1) * PT],
                        xtj[:, d, :],
                        start=(d == 0),
                        stop=(d == EC - 1),
                    )

            for j in range(JC - 2):
                xtj = xts_sb if j == 0 else xtjs[j]
                if j > 0:
                    nc.sync.dma_start(xtj, xTb[j])
                for i in range(IT):
                    ps = spsum.tile([PT, 512], F32, tag="Sp", name="Sp")
                    mm_group(ps, i, xtj)
                    softmax_step(ps, i, j)
                if j == 0:
                    xspool_cm.__exit__(None, None, None)

            # Chunks 6-7 are processed i0/i1-first so their finalize chain
            # (which gates phase C's first transposes) completes ~2 PE groups
            # before B's compute ends. i3's last group goes to a dedicated
            # PSUM bank so its deferred exp gates nothing in phase C.
            nc.sync.dma_start(xtjs[JC - 2], xTb[JC - 2])
            nc.sync.dma_start(xtjs[JC - 1], xTb[JC - 1])
            xt6, xt7 = xtjs[JC - 2], xtjs[JC - 1]

            def subchain(i):
                # gk = ck * (1/Z); first two diag tiles built inline on DVE.
                nc.vector.tensor_tensor(
                    out=gk[i], in0=zpart[i], in1=ck[i], op=ALU.mult
                )
                nc.vector.reduce_sum(out=zsum[i], in_=gk[i], axis=AX.X)
                nc.vector.reciprocal(rz[i], zsum[i])
                nc.vector.tensor_scalar_mul(gk[i], ck[i], rz[i])
                for k in (0, 1):
                    nc.vector.tensor_scalar_mul(
                        diag[:, i, k, :], ident, gk[i][:, k : k + 1]
                    )

            for i in (0, 1):
                ps = spsum.tile([PT, 512], F32, tag="Sp", name="Sp")
                mm_group(ps, i, xt6)
                softmax_step(ps, i, JC - 2)
            ps70 = spsum.tile([PT, 512], F32, tag="Sp", name="Sp")
            mm_group(ps70, 0, xt7)
            ps71 = spsum.tile([PT, 512], F32, tag="Sp", name="Sp")
            mm_group(ps71, 1, xt7)
            softmax_step(ps70, 0, JC - 1)
            nc.scalar.activation(
                out=ck[0], in_=nmk[0], func=AF.Exp,
                bias=nmk[0][:, JC - 1 : JC], scale=-1.0,
            )
            softmax_step(ps71, 1, JC - 1)
            nc.scalar.activation(
                out=ck[1], in_=nmk[1], func=AF.Exp,
                bias=nmk[1][:, JC - 1 : JC], scale=-1.0,
            )
            subchain(0)
            subchain(1)
            for i in (2, 3):
                ps = spsum.tile([PT, 512], F32, tag="Sp", name="Sp")
                mm_group(ps, i, xt6)
                softmax_step(ps, i, JC - 2)
            ps72 = spsum.tile([PT, 512], F32, tag="Sp", name="Sp")
            mm_group(ps72, 2, xt7)
            softmax_step(ps72, 2, JC - 1)
            # pad tiles reserve banks 4-6 (never written) so the final i3
            # group lands on bank 7, which phase C never reallocates; its
            # exp can then be deferred into phase C's queues safely.
            for pb in range(3):
                padpool.tile([PT, 512], F32, tag=f"pad{pb}", name=f"pad{pb}")
            ps73 = padpool.tile([PT, 512], F32, tag="Spz", name="Spz")
            mm_group(ps73, 3, xt7)

            def red3min3():
                nc.vector.reduce_max(
                    out=tmx[3][:, JC - 1 : JC], in_=ps73, axis=AX.X, negate=True
                )
                nc.vector.tensor_tensor(
                    out=nmk[3][:, JC - 1 : JC], in0=nmk[3][:, JC - 2 : JC - 1],
                    in1=tmx[3][:, JC - 1 : JC], op=ALU.min,
                )

            def exp3():
                nc.scalar.activation(
                    out=E_bf[3][:, (JC - 1) * 512 : JC * 512],
                    in_=ps73, func=AF.Exp,
                    bias=nmk[3][:, JC - 1 : JC], scale=1.0,
                    accum_out=zpart[3][:, JC - 1 : JC],
                )

            def dgk01(k):
                def emit():
                    nc.vector.tensor_scalar_mul(
                        diag[:, 0, k, :], ident, gk[0][:, k : k + 1]
                    )
                    nc.scalar.activation(
                        diag[:, 1, k, :], ident, func=AF.Copy,
                        scale=gk[1][:, k : k + 1],
                    )
                return emit

            def fin23a():
                for i in (2, 3):
                    nc.scalar.activation(
                        out=ck[i], in_=nmk[i], func=AF.Exp,
                        bias=nmk[i][:, JC - 1 : JC], scale=-1.0,
                    )
                for i in (2, 3):
                    nc.vector.tensor_tensor(
                        out=gk[i], in0=zpart[i], in1=ck[i], op=ALU.mult
                    )
                for i in (2, 3):
                    nc.vector.reduce_sum(out=zsum[i], in_=gk[i], axis=AX.X)
                for i in (2, 3):
                    nc.vector.reciprocal(rz[i], zsum[i])
                for i in (2, 3):
                    nc.vector.tensor_scalar_mul(gk[i], ck[i], rz[i])

            def dg23(k0, k1):
                def emit():
                    for k in range(k0, k1):
                        nc.vector.tensor_scalar_mul(
                            diag[:, 2, k, :], ident, gk[2][:, k : k + 1]
                        )
                        nc.scalar.activation(
                            diag[:, 3, k, :], ident, func=AF.Copy,
                            scale=gk[3][:, k : k + 1],
                        )
                return emit

            bpend.append(red3min3)
            bpend.append(exp3)
            for k in range(2, JC):
                bpend.append(dgk01(k))
            bpend.append(fin23a)
            bpend.append(dg23(0, 4))
            bpend.append(dg23(4, JC))

        # ---- Phase C: out = P @ x with x fully resident in SBUF.
        # xb reuses the phase-A weight pool's address range; its triggers sit
        # on the sync ring behind the phase-B stream.
        xbpool = big.enter_context(tc.tile_pool(name="xbpool", bufs=1))
        xb_sb = xbpool.tile([PT, JT, D], BF16)
        xb_r = xb.rearrange("(t p) d -> p t d", p=PT)
        for g in range(8):
            nc.sync.dma_start(
                xb_sb[:, 4 * g : 4 * g + 4, :], xb_r[:, 4 * g : 4 * g + 4, :]
            )
        etpool = big.enter_context(tc.tile_pool(name="etpool", bufs=3))
        ocopy = big.enter_context(tc.tile_pool(name="ocopy", bufs=4))
        # opsum allocated before tpsum: oacc lands on the banks whose phase-B
        # WARs clear first (their mms only start LOOKP pairs in), tpsum gets
        # the late-released + fresh banks so the first transposes don't wait
        # on i2/i3's last exps.
        opsum = big.enter_context(tc.tile_pool(name="opsum", bufs=1, space="PSUM"))
        tpsum = big.enter_context(tc.tile_pool(name="tpsum", bufs=3, space="PSUM"))
        # accumulators are shared by both passes (keyed by position kp);
        # pass 1's first start=True matmuls depend on pass 0's drain copies,
        # which are emitted interleaved into pass 1's first pairs.
        oacc = {
            (kp, dn): opsum.tile([PT, 512], F32, tag=f"o{kp}_{dn}", name=f"o{kp}_{dn}")
            for kp in range(2)
            for dn in range(2)
        }

        def drain_item(kp, dn, i):
            def emit():
                ot = ocopy.tile([PT, 512], F32, tag="ot", name="ot")
                if dn == 0:
                    nc.vector.tensor_copy(ot, oacc[(kp, dn)])
                    nc.sync.dma_start(out[i * PT : (i + 1) * PT, 0:512], ot)
                else:
                    nc.scalar.activation(ot, oacc[(kp, dn)], func=AF.Copy)
                    nc.scalar.dma_start(out[i * PT : (i + 1) * PT, 512:1024], ot)

            return emit

        NP = JT // 2  # jt pairs
        LOOKP = 2
        pending = bpend
        for pi, ii in enumerate(((0, 1), (2, 3))):
            ets = {}
            for pv in range(NP + LOOKP):
                if pv < NP:
                    # "transpose" = E_tile.T @ diag(g): per-row softmax scale
                    # applied for free by the mandatory transpose. Two jt per
                    # pst bank -> one PSUM->SBUF copy per pair.
                    pst = tpsum.tile([PT, 512], F32, tag="tp", name="pst")
                    for kp, i in enumerate(ii):
                        for s in range(2):
                            jt = 2 * pv + s
                            nc.tensor.matmul(
                                pst[:, (2 * kp + s) * PT : (2 * kp + s + 1) * PT],
                                E_bf[i][:, jt * PT : (jt + 1) * PT],
                                diag[:, i, jt // 4, :],
                                start=True,
                                stop=True,
                                skip_group_check=True,
                            )
                    et = etpool.tile([PT, 512], BF16, tag="et", name="et")
                    if pv % 2 == 0:
                        nc.vector.tensor_copy(et, pst)
                    else:
                        nc.scalar.activation(et, pst, func=AF.Copy)
                    ets[pv % 3] = et
                    # pops start at pv 1 so pair 0's et copy isn't queued
                    # behind the deferred DVE work; two per pair so all of
                    # the previous pass's drains are emitted before this
                    # pass's first start=True matmuls reuse the banks.
                    if pv >= 1:
                        for _ in range(2):
                            if pending:
                                pending.pop(0)()
                if pv >= LOOKP:
                    p = pv - LOOKP
                    for s in range(2):
                        jt = 2 * p + s
                        for kp, i in enumerate(ii):
                            for dn in range(2):
                                nc.tensor.matmul(
                                    oacc[(kp, dn)],
                                    ets[p % 3][
                                        :, (2 * kp + s) * PT : (2 * kp + s + 1) * PT
                                    ],
                                    xb_sb[:, jt, dn * 512 : (dn + 1) * 512],
                                    start=(jt == 0),
                                    stop=(jt == JT - 1),
                                )
            drains = [
                drain_item(kp, dn, i)
                for kp, i in enumerate(ii)
                for dn in range(2)
            ]
            if pi == 0:
                pending = drains
            else:
                for d in drains:
                    d()


def build():
    nc = bacc.Bacc(
        "TRN2",
        target_bir_lowering=False,
        debug=False,
        enable_asserts=False,
        num_devices=NCORES,
    )
    aps = {
        "xTb": nc.dram_tensor("xTb", [JC, PT, EC, 512], F32R, kind="ExternalInput").ap(),
        "xTs": nc.dram_tensor("xTs", [D, R], F32R, kind="ExternalInput").ap(),
        "mw": nc.dram_tensor("mw", [D, D], F32R, kind="ExternalInput").ap(),
        "cw": nc.dram_tensor("cw", [PT, EC], F32, kind="ExternalInput").ap(),
        "xb": nc.dram_tensor("xb", [N, D], BF16, kind="ExternalInput").ap(),
        "out": nc.dram_tensor("out", [R, D], F32, kind="ExternalOutput").ap(),
    }
    with tile.TileContext(nc) as tc:
        _emit(nc, tc, aps)
    nc.compile()
    return nc


_NC_CACHE = None
LAST_RESULTS = None


def _get_nc():
    global _NC_CACHE
    if _NC_CACHE is None:
        _NC_CACHE = build()
    return _NC_CACHE


def make_in_maps(x, Wq, bq, Wk):
    x = np.ascontiguousarray(np.asarray(x, dtype=np.float32))
    xT = np.ascontiguousarray(x.T)
    # xTb[j, p, e, n] = xT[e*128 + p, j*512 + n]: per-(j,p) contiguous 16KB
    # blocks so the phase-B stream DMAs at full descriptor size.
    xTb = np.ascontiguousarray(
        xT.reshape(EC, PT, JC, 512).transpose(2, 1, 0, 3)
    )
    wk64 = np.asarray(Wk, dtype=np.float64)
    mw = np.ascontiguousarray(
        (np.asarray(Wq, dtype=np.float64).T @ wk64).astype(np.float32)
    )
    # cw[p, e] = c[e*128 + p]: per-partition bias column for the tT copies.
    cw = np.ascontiguousarray(
        (np.asarray(bq, dtype=np.float64) @ wk64)
        .astype(np.float32)
        .reshape(EC, PT)
        .T
    )
    xb = x.astype(ml_dtypes.bfloat16)
    in_maps = []
    for c in range(NCORES):
        # Each core processes key chunks in rotated order [c, c+1, ..]: its
        # own query slice xTs doubles as stream position 0 (already in SBUF
        # when phase B starts), so xTb and xb are rotated to match. The
        # rotation permutes softmax terms and P@x rows consistently; the
        # output rows (queries) are unaffected.
        in_maps.append(
            {
                "xTb": np.ascontiguousarray(
                    np.concatenate([xTb[c:], xTb[:c]], axis=0)
                ),
                "xTs": np.ascontiguousarray(xT[:, c * R : (c + 1) * R]),
                "mw": mw,
                "cw": cw,
                "xb": np.ascontiguousarray(np.roll(xb, -512 * c, axis=0)),
            }
        )
    return in_maps


def kernel(x, Wq, bq, Wk, bk):
    # bk only shifts each score row by a constant, which softmax cancels.
    del bk
    in_maps = make_in_maps(x, Wq, bq, Wk)
    nc = _get_nc()
    kwargs = {}
    if os.environ.get("K_TRACE_DIR"):
        import tempfile

        kwargs["tmpdir"] = tempfile.mkdtemp(dir=os.environ["K_TRACE_DIR"])
    res = run_bass_kernel_spmd(nc, in_maps, core_ids=list(range(NCORES)), **kwargs)
    global LAST_RESULTS
    LAST_RESULTS = res
    return np.concatenate(
        [np.asarray(res.results[c]["out"], dtype=np.float32) for c in range(NCORES)],
        axis=0,
    )


# revision 17
# speedup vs baseline: 1.1970x; 1.0090x over previous
"""Trainium2 Bass kernel for CLIP attention pooling.

Reference computation (N=4096, D=1024, fp32):
    q = x @ Wq.T + bq
    k = x @ Wk.T + bk
    attn = softmax(q @ k.T, axis=-1)
    out = attn @ x

Math notes:
  * scores = q @ k.T = q @ Wk @ x.T + (q.bk) 1^T. The (q.bk) term is
    constant along the softmax axis, so bk never needs to be computed.
  * q @ Wk = x @ (Wq.T @ Wk) + bq @ Wk: both projections fold into one
    matrix M = Wq.T @ Wk and a row c = bq @ Wk (host-precomputed).
  * Per core (512 query rows):
        tT = M^T . xs^T + c          [D, 512]   (transposed layout)
        S  = t . x^T                 [512, 4096]
        P  = softmax(S)              (online, running-max)
        out = P @ x                  [512, 1024]

Schedule (v3):
  * phase A: M chunks stream on the sync HWDGE ring, xs chunks on the
    scalar ring (independent trigger FIFOs); e-outer over 8 PSUM banks;
    the bias row enters via a K=1 (c x ones) matmul per bank. The
    phase-B chunk-0 stream DMA is interleaved into the M trigger
    sequence so it lands just before phase A's compute finishes.
  * phase B: x^T streams in 512-column chunks (double-buffered).
    Softmax is ONLINE: per (i, chunk) a running negated max is
    maintained on DVE straight out of PSUM, and ACT applies
    exp(PSUM - runmax) directly into bf16 E with accum_out collecting
    per-chunk partial sums. No S buffer exists.
  * after the last chunk per i: c_k = exp(m_k - m_final), Z = sum_k
    z_k c_k, g_k = c_k / Z. The g_k become 32 per-(i,chunk) DIAGONAL
    matrices (bf16), built i-grouped alternating DVE/ACT; phase C
    "transposes" are plain matmuls E_tile @ diag(g) so the softmax
    normalization and running-max corrections ride the mandatory
    transpose for free.
  * phase C: x (bf16) is fully resident in SBUF (8MB, loaded on the
    sync ring behind the phase-B stream; its buffer aliases only the
    phase-A weight pool so the triggers fire as soon as the ring
    drains). Passes over i-tiles {0,1}/{2}/{3}: pass 0 needs only the
    first two diags so its transposes start while phase B's last
    matmuls still run; the last pass leaves just 512KB of output for
    the tail. A single shared PSUM transpose pool avoids cross-pass
    bank WARs (pass 1 lands on never-used banks). Output copy/DMA
    pairs alternate DVE/ACT engines and sync/scalar DMA rings.
"""

import os
from contextlib import ExitStack

import numpy as np
import ml_dtypes

import concourse.bass as bass
import concourse.mybir as mybir
import concourse.tile as tile
from concourse import bacc
from concourse.bass_utils import run_bass_kernel_spmd
from concourse.masks import make_identity

N, D = 4096, 1024
NCORES = 8
R = N // NCORES  # 512 query rows per core
PT = 128  # partition tile
EC = D // PT  # 8 contraction chunks of the model dim
IT = R // PT  # 4 query tiles per core
JC = N // 512  # 8 key chunks of 512
JT = N // PT  # 32 key tiles of 128

F32 = mybir.dt.float32
F32R = mybir.dt.float32r
BF16 = mybir.dt.bfloat16
AX = mybir.AxisListType
AF = mybir.ActivationFunctionType
ALU = mybir.AluOpType

PASSES = ((0, 1), (2,), (3,))


def _emit(nc: bass.Bass, tc: tile.TileContext, aps: dict):
    xTb, xTs, mw, cw, xb, out = (
        aps["xTb"], aps["xTs"], aps["mw"], aps["cw"],
        aps["xb"], aps["out"],
    )

    with ExitStack() as big:
        persist = big.enter_context(tc.tile_pool(name="persist", bufs=1))

        ident = persist.tile([PT, PT], BF16)
        make_identity(nc, ident)
        c_sb = persist.tile([PT, EC], F32)

        tT_sb = persist.tile([PT, EC, R], F32R)
        E_bf = [persist.tile([PT, N], BF16, name=f"E{i}") for i in range(IT)]
        nmk = [persist.tile([PT, JC], F32, name=f"nmk{i}") for i in range(IT)]
        tmx = [persist.tile([PT, JC], F32, name=f"tmx{i}") for i in range(IT)]
        zpart = [persist.tile([PT, JC], F32, name=f"zp{i}") for i in range(IT)]
        ck = [persist.tile([PT, JC], F32, name=f"ck{i}") for i in range(IT)]
        gk = [persist.tile([PT, JC], F32, name=f"gk{i}") for i in range(IT)]
        zsum = [persist.tile([PT, 1], F32, name=f"z{i}") for i in range(IT)]
        rz = [persist.tile([PT, 1], F32, name=f"rz{i}") for i in range(IT)]
        diag = persist.tile([PT, IT, JC, PT], BF16)

        # opened before wpool so its addresses never overlap the weights;
        # the early stream triggers can then issue during phase A.
        xtpool = big.enter_context(tc.tile_pool(name="xtpool", bufs=4))
        xtjs = {}
        for j in range(1, JC):
            xtjs[j] = xtpool.tile([PT, EC, 512], F32R, tag="xtj", name="xtj")

        # xs in its own pool (opened after xtpool so pools unwind LIFO):
        # stream position 0 of phase B reads it directly - each core's query
        # slice IS its own key chunk - so it is only released after that;
        # the xb buffer then aliases it + wpool.
        xspool_cm = tc.tile_pool(name="xspool", bufs=1)
        xspool = xspool_cm.__enter__()
        xts_sb = xspool.tile([PT, EC, R], F32R)

        # ---- Phase A: tT = M^T.xs^T + c  (transposed layout)
        with ExitStack() as pha:
            wpool = pha.enter_context(tc.tile_pool(name="wpool", bufs=1))
            apsum = pha.enter_context(tc.tile_pool(name="apsum", bufs=1, space="PSUM"))

            m_sb = wpool.tile([PT, EC, D], F32R)

            m_r = mw.rearrange("(t p) d -> p t d", p=PT)
            xTs_r = xTs.rearrange("(t p) i -> p t i", p=PT)
            # M rides the sync HWDGE ring, xs + bias the scalar ring: the
            # trigger FIFOs are independent and the SDMA engines round-robin
            # between them. Chunk 0 of the phase-B stream is slotted in
            # before the last two M chunks: phase A's compute tail covers it.
            nc.sync.dma_start(m_sb[:, 0, 0:256], m_r[:, 0, 0:256])
            nc.scalar.dma_start(xts_sb[:, 0, :], xTs_r[:, 0, :])
            nc.sync.dma_start(m_sb[:, 0, 256:D], m_r[:, 0, 256:D])
            nc.scalar.dma_start(xts_sb[:, 1, :], xTs_r[:, 1, :])
            nc.scalar.dma_start(c_sb, cw)
            for e in range(1, EC):
                nc.sync.dma_start(m_sb[:, e, :], m_r[:, e, :])
            for e in range(2, EC):
                nc.scalar.dma_start(xts_sb[:, e, :], xTs_r[:, e, :])

            tps = [
                apsum.tile([PT, R], F32, tag=f"tp{d}", name=f"tp{d}")
                for d in range(EC)
            ]
            for e in range(EC):
                for d in range(EC):
                    nc.tensor.matmul(
                        tps[d],
                        m_sb[:, e, d * PT : (d + 1) * PT],
                        xts_sb[:, e, :],
                        start=(e == 0),
                        stop=(e == EC - 1),
                    )
            for d in range(EC):
                # bias folded into the PSUM->SBUF copy: tT[d_blk,:] += c[d_blk]
                if d % 2 == 0:
                    nc.vector.tensor_scalar_add(
                        tT_sb[:, d, :], tps[d], c_sb[:, d : d + 1]
                    )
                else:
                    nc.scalar.activation(
                        tT_sb[:, d, :], tps[d], func=AF.Identity,
                        bias=c_sb[:, d : d + 1],
                    )

        # ---- Phase B: S chunks in PSUM + online softmax straight to E.
        def softmax_step(ps, i, j):
            if j == 0:
                nc.vector.reduce_max(
                    out=nmk[i][:, 0:1], in_=ps, axis=AX.X, negate=True
                )
            else:
                nc.vector.reduce_max(
                    out=tmx[i][:, j : j + 1], in_=ps, axis=AX.X, negate=True
                )
                nc.vector.tensor_tensor(
                    out=nmk[i][:, j : j + 1],
                    in0=nmk[i][:, j - 1 : j],
                    in1=tmx[i][:, j : j + 1],
                    op=ALU.min,
                )
            nc.scalar.activation(
                out=E_bf[i][:, j * 512 : (j + 1) * 512],
                in_=ps,
                func=AF.Exp,
                bias=nmk[i][:, j : j + 1],
                scale=1.0,
                accum_out=zpart[i][:, j : j + 1],
            )

        def finalize_pair(ia, ib):
            # c_k = exp(m_k - m_last), Z = sum z_k c_k, g = c_k/Z; then the
            # per-chunk diag(g) tiles, k-ordered round-robin across DVE/ACT
            # so both i-tiles' early-k diags finish first, in parallel.
            for i in (ia, ib):
                nc.scalar.activation(
                    out=ck[i],
                    in_=nmk[i],
                    func=AF.Exp,
                    bias=nmk[i][:, JC - 1 : JC],
                    scale=-1.0,
                )
            for i in (ia, ib):
                nc.vector.tensor_tensor(
                    out=gk[i], in0=zpart[i], in1=ck[i], op=ALU.mult
                )
            for i in (ia, ib):
                nc.vector.reduce_sum(out=zsum[i], in_=gk[i], axis=AX.X)
            for i in (ia, ib):
                nc.vector.reciprocal(rz[i], zsum[i])
            for i in (ia, ib):
                nc.vector.tensor_scalar_mul(gk[i], ck[i], rz[i])
            for k in range(JC):
                dve_i = ia if k % 2 == 0 else ib
                act_i = ib if k % 2 == 0 else ia
                nc.vector.tensor_scalar_mul(
                    diag[:, dve_i, k, :], ident, gk[dve_i][:, k : k + 1]
                )
                nc.scalar.activation(
                    diag[:, act_i, k, :],
                    ident,
                    func=AF.Copy,
                    scale=gk[act_i][:, k : k + 1],
                )

        bpend = []
        with ExitStack() as phb:
            spsum = phb.enter_context(tc.tile_pool(name="spsum", bufs=4, space="PSUM"))
            padpool = phb.enter_context(
                tc.tile_pool(name="padpool", bufs=1, space="PSUM")
            )
            def mm_group(ps, i, xtj):
                for d in range(EC):
                    nc.tensor.matmul(
                        ps,
                        tT_sb[:, d, i * PT : (i + 1) * PT],
                        xtj[:, d, :],
                        start=(d == 0),
                        stop=(d == EC - 1),
                    )

            for j in range(JC - 2):
                xtj = xts_sb if j == 0 else xtjs[j]
                if j > 0:
                    nc.sync.dma_start(xtj, xTb[j])
                for i in range(IT):
                    ps = spsum.tile([PT, 512], F32, tag="Sp", name="Sp")
                    mm_group(ps, i, xtj)
                    softmax_step(ps, i, j)
                if j == 0:
                    xspool_cm.__exit__(None, None, None)

            # Chunks 6-7 are processed i0/i1-first so their finalize chain
            # (which gates phase C's first transposes) completes ~2 PE groups
            # before B's compute ends. i3's last group goes to a dedicated
            # PSUM bank so its deferred exp gates nothing in phase C.
            nc.sync.dma_start(xtjs[JC - 2], xTb[JC - 2])
            nc.sync.dma_start(xtjs[JC - 1], xTb[JC - 1])
            xt6, xt7 = xtjs[JC - 2], xtjs[JC - 1]

            def subchain(i):
                # gk = ck * (1/Z); first two diag tiles built inline on DVE.
                nc.vector.tensor_tensor(
                    out=gk[i], in0=zpart[i], in1=ck[i], op=ALU.mult
                )
                nc.vector.reduce_sum(out=zsum[i], in_=gk[i], axis=AX.X)
                nc.vector.reciprocal(rz[i], zsum[i])
                nc.vector.tensor_scalar_mul(gk[i], ck[i], rz[i])
                for k in (0, 1):
                    nc.vector.tensor_scalar_mul(
                        diag[:, i, k, :], ident, gk[i][:, k : k + 1]
                    )

            for i in (0, 1):
                ps = spsum.tile([PT, 512], F32, tag="Sp", name="Sp")
                mm_group(ps, i, xt6)
                softmax_step(ps, i, JC - 2)
            ps70 = spsum.tile([PT, 512], F32, tag="Sp", name="Sp")
            mm_group(ps70, 0, xt7)
            ps71 = spsum.tile([PT, 512], F32, tag="Sp", name="Sp")
            mm_group(ps71, 1, xt7)
            softmax_step(ps70, 0, JC - 1)
            nc.scalar.activation(
                out=ck[0], in_=nmk[0], func=AF.Exp,
                bias=nmk[0][:, JC - 1 : JC], scale=-1.0,
            )
            softmax_step(ps71, 1, JC - 1)
            nc.scalar.activation(
                out=ck[1], in_=nmk[1], func=AF.Exp,
                bias=nmk[1][:, JC - 1 : JC], scale=-1.0,
            )
            subchain(0)
            subchain(1)
            for i in (2, 3):
                ps = spsum.tile([PT, 512], F32, tag="Sp", name="Sp")
                mm_group(ps, i, xt6)
                softmax_step(ps, i, JC - 2)
            ps72 = spsum.tile([PT, 512], F32, tag="Sp", name="Sp")
            mm_group(ps72, 2, xt7)
            softmax_step(ps72, 2, JC - 1)
            # pad tiles reserve banks 4-6 (never written) so the final i3
            # group lands on bank 7, which phase C never reallocates; its
            # exp can then be deferred into phase C's queues safely.
            for pb in range(3):
                padpool.tile([PT, 512], F32, tag=f"pad{pb}", name=f"pad{pb}")
            ps73 = padpool.tile([PT, 512], F32, tag="Spz", name="Spz")
            mm_group(ps73, 3, xt7)

            def red3min3():
                nc.vector.reduce_max(
                    out=tmx[3][:, JC - 1 : JC], in_=ps73, axis=AX.X, negate=True
                )
                nc.vector.tensor_tensor(
                    out=nmk[3][:, JC - 1 : JC], in0=nmk[3][:, JC - 2 : JC - 1],
                    in1=tmx[3][:, JC - 1 : JC], op=ALU.min,
                )

            def exp3():
                nc.scalar.activation(
                    out=E_bf[3][:, (JC - 1) * 512 : JC * 512],
                    in_=ps73, func=AF.Exp,
                    bias=nmk[3][:, JC - 1 : JC], scale=1.0,
                    accum_out=zpart[3][:, JC - 1 : JC],
                )

            def dgk01(k):
                def emit():
                    nc.vector.tensor_scalar_mul(
                        diag[:, 0, k, :], ident, gk[0][:, k : k + 1]
                    )
                    nc.scalar.activation(
                        diag[:, 1, k, :], ident, func=AF.Copy,
                        scale=gk[1][:, k : k + 1],
                    )
                return emit

            def fin23a():
                for i in (2, 3):
                    nc.scalar.activation(
                        out=ck[i], in_=nmk[i], func=AF.Exp,
                        bias=nmk[i][:, JC - 1 : JC], scale=-1.0,
                    )
                for i in (2, 3):
                    nc.vector.tensor_tensor(
                        out=gk[i], in0=zpart[i], in1=ck[i], op=ALU.mult
                    )
                for i in (2, 3):
                    nc.vector.reduce_sum(out=zsum[i], in_=gk[i], axis=AX.X)
                for i in (2, 3):
                    nc.vector.reciprocal(rz[i], zsum[i])
                for i in (2, 3):
                    nc.vector.tensor_scalar_mul(gk[i], ck[i], rz[i])

            def dg23(k0, k1):
                def emit():
                    for k in range(k0, k1):
                        nc.vector.tensor_scalar_mul(
                            diag[:, 2, k, :], ident, gk[2][:, k : k + 1]
                        )
                        nc.scalar.activation(
                            diag[:, 3, k, :], ident, func=AF.Copy,
                            scale=gk[3][:, k : k + 1],
                        )
                return emit

            bpend.append(red3min3)
            bpend.append(exp3)
            for k in range(2, JC):
                bpend.append(dgk01(k))
            bpend.append(fin23a)
            bpend.append(dg23(0, 4))
            bpend.append(dg23(4, JC))

        # ---- Phase C: out = P @ x with x fully resident in SBUF.
        # xb reuses the phase-A weight pool's address range; its triggers sit
        # on the sync ring behind the phase-B stream.
        xbpool = big.enter_context(tc.tile_pool(name="xbpool", bufs=1))
        xb_sb = xbpool.tile([PT, JT, D], BF16)
        nc.sync.dma_start(xb_sb[:, 0 : JT // 2, :], xb[:, 0 : JT // 2, :])
        nc.sync.dma_start(xb_sb[:, JT // 2 : JT, :], xb[:, JT // 2 : JT, :])
        etpool = big.enter_context(tc.tile_pool(name="etpool", bufs=3))
        ocopy = big.enter_context(tc.tile_pool(name="ocopy", bufs=4))
        # opsum allocated before tpsum: oacc lands on the banks whose phase-B
        # WARs clear first (their mms only start LOOKP pairs in), tpsum gets
        # the late-released + fresh banks so the first transposes don't wait
        # on i2/i3's last exps.
        opsum = big.enter_context(tc.tile_pool(name="opsum", bufs=1, space="PSUM"))
        tpsum = big.enter_context(tc.tile_pool(name="tpsum", bufs=3, space="PSUM"))
        # accumulators are shared by both passes (keyed by position kp);
        # pass 1's first start=True matmuls depend on pass 0's drain copies,
        # which are emitted interleaved into pass 1's first pairs.
        oacc = {
            (kp, dn): opsum.tile([PT, 512], F32, tag=f"o{kp}_{dn}", name=f"o{kp}_{dn}")
            for kp in range(2)
            for dn in range(2)
        }

        def drain_item(kp, dn, i):
            def emit():
                ot = ocopy.tile([PT, 512], F32, tag="ot", name="ot")
                if dn == 0:
                    nc.vector.tensor_copy(ot, oacc[(kp, dn)])
                    nc.sync.dma_start(out[i * PT : (i + 1) * PT, 0:512], ot)
                else:
                    nc.scalar.activation(ot, oacc[(kp, dn)], func=AF.Copy)
                    nc.scalar.dma_start(out[i * PT : (i + 1) * PT, 512:1024], ot)

            return emit

        NP = JT // 2  # jt pairs
        LOOKP = 2
        pending = bpend
        for pi, ii in enumerate(((0, 1), (2, 3))):
            ets = {}
            for pv in range(NP + LOOKP):
                if pv < NP:
                    # "transpose" = E_tile.T @ diag(g): per-row softmax scale
                    # applied for free by the mandatory transpose. Two jt per
                    # pst bank -> one PSUM->SBUF copy per pair.
                    pst = tpsum.tile([PT, 512], F32, tag="tp", name="pst")
                    for kp, i in enumerate(ii):
                        for s in range(2):
                            jt = 2 * pv + s
                            nc.tensor.matmul(
                                pst[:, (2 * kp + s) * PT : (2 * kp + s + 1) * PT],
                                E_bf[i][:, jt * PT : (jt + 1) * PT],
                                diag[:, i, jt // 4, :],
                                start=True,
                                stop=True,
                                skip_group_check=True,
                            )
                    et = etpool.tile([PT, 512], BF16, tag="et", name="et")
                    if pv % 2 == 0:
                        nc.vector.tensor_copy(et, pst)
                    else:
                        nc.scalar.activation(et, pst, func=AF.Copy)
                    ets[pv % 3] = et
                    # pops start at pv 1 so pair 0's et copy isn't queued
                    # behind the deferred DVE work; two per pair so all of
                    # the previous pass's drains are emitted before this
                    # pass's first start=True matmuls reuse the banks.
                    if pv >= 1:
                        for _ in range(2):
                            if pending:
                                pending.pop(0)()
                if pv >= LOOKP:
                    p = pv - LOOKP
                    for s in range(2):
                        jt = 2 * p + s
                        for kp, i in enumerate(ii):
                            for dn in range(2):
                                nc.tensor.matmul(
                                    oacc[(kp, dn)],
                                    ets[p % 3][
                                        :, (2 * kp + s) * PT : (2 * kp + s + 1) * PT
                                    ],
                                    xb_sb[:, jt, dn * 512 : (dn + 1) * 512],
                                    start=(jt == 0),
                                    stop=(jt == JT - 1),
                                )
            drains = [
                drain_item(kp, dn, i)
                for kp, i in enumerate(ii)
                for dn in range(2)
            ]
            if pi == 0:
                pending = drains
            else:
                for d in drains:
                    d()


def build():
    nc = bacc.Bacc(
        "TRN2",
        target_bir_lowering=False,
        debug=False,
        enable_asserts=False,
        num_devices=NCORES,
    )
    aps = {
        "xTb": nc.dram_tensor("xTb", [JC, PT, EC, 512], F32R, kind="ExternalInput").ap(),
        "xTs": nc.dram_tensor("xTs", [D, R], F32R, kind="ExternalInput").ap(),
        "mw": nc.dram_tensor("mw", [D, D], F32R, kind="ExternalInput").ap(),
        "cw": nc.dram_tensor("cw", [PT, EC], F32, kind="ExternalInput").ap(),
        "xb": nc.dram_tensor("xb", [PT, JT, D], BF16, kind="ExternalInput").ap(),
        "out": nc.dram_tensor("out", [R, D], F32, kind="ExternalOutput").ap(),
    }
    with tile.TileContext(nc) as tc:
        _emit(nc, tc, aps)
    nc.compile()
    return nc


_NC_CACHE = None
LAST_RESULTS = None


def _get_nc():
    global _NC_CACHE
    if _NC_CACHE is None:
        _NC_CACHE = build()
    return _NC_CACHE


def make_in_maps(x, Wq, bq, Wk):
    x = np.ascontiguousarray(np.asarray(x, dtype=np.float32))
    xT = np.ascontiguousarray(x.T)
    # xTb[j, p, e, n] = xT[e*128 + p, j*512 + n]: per-(j,p) contiguous 16KB
    # blocks so the phase-B stream DMAs at full descriptor size.
    xTb = np.ascontiguousarray(
        xT.reshape(EC, PT, JC, 512).transpose(2, 1, 0, 3)
    )
    wk64 = np.asarray(Wk, dtype=np.float64)
    mw = np.ascontiguousarray(
        (np.asarray(Wq, dtype=np.float64).T @ wk64).astype(np.float32)
    )
    # cw[p, e] = c[e*128 + p]: per-partition bias column for the tT copies.
    cw = np.ascontiguousarray(
        (np.asarray(bq, dtype=np.float64) @ wk64)
        .astype(np.float32)
        .reshape(EC, PT)
        .T
    )
    xb = x.astype(ml_dtypes.bfloat16)
    in_maps = []
    for c in range(NCORES):
        # Each core processes key chunks in rotated order [c, c+1, ..]: its
        # own query slice xTs doubles as stream position 0 (already in SBUF
        # when phase B starts), so xTb and xb are rotated to match. The
        # rotation permutes softmax terms and P@x rows consistently; the
        # output rows (queries) are unaffected.
        in_maps.append(
            {
                "xTb": np.ascontiguousarray(
                    np.concatenate([xTb[c:], xTb[:c]], axis=0)
                ),
                "xTs": np.ascontiguousarray(xT[:, c * R : (c + 1) * R]),
                "mw": mw,
                "cw": cw,
                "xb": np.ascontiguousarray(
                    np.roll(xb, -512 * c, axis=0)
                    .reshape(JT, PT, D)
                    .transpose(1, 0, 2)
                ),
            }
        )
    return in_maps


def kernel(x, Wq, bq, Wk, bk):
    # bk only shifts each score row by a constant, which softmax cancels.
    del bk
    in_maps = make_in_maps(x, Wq, bq, Wk)
    nc = _get_nc()
    kwargs = {}
    if os.environ.get("K_TRACE_DIR"):
        import tempfile

        kwargs["tmpdir"] = tempfile.mkdtemp(dir=os.environ["K_TRACE_DIR"])
    res = run_bass_kernel_spmd(nc, in_maps, core_ids=list(range(NCORES)), **kwargs)
    global LAST_RESULTS
    LAST_RESULTS = res
    return np.concatenate(
        [np.asarray(res.results[c]["out"], dtype=np.float32) for c in range(NCORES)],
        axis=0,
    )


# revision 21
# speedup vs baseline: 1.2102x; 1.0111x over previous
"""Trainium2 Bass kernel for CLIP attention pooling.

Reference computation (N=4096, D=1024, fp32):
    q = x @ Wq.T + bq
    k = x @ Wk.T + bk
    attn = softmax(q @ k.T, axis=-1)
    out = attn @ x

Math notes:
  * scores = q @ k.T = q @ Wk @ x.T + (q.bk) 1^T. The (q.bk) term is
    constant along the softmax axis, so bk never needs to be computed.
  * q @ Wk = x @ (Wq.T @ Wk) + bq @ Wk: both projections fold into one
    matrix M = Wq.T @ Wk and a row c = bq @ Wk (host-precomputed).
  * Per core (512 query rows):
        tT = M^T . xs^T + c          [D, 512]   (transposed layout)
        S  = t . x^T                 [512, 4096]
        P  = softmax(S)              (online, running-max)
        out = P @ x                  [512, 1024]

Schedule (v3):
  * phase A: M chunks stream on the sync HWDGE ring, xs chunks on the
    scalar ring (independent trigger FIFOs); e-outer over 8 PSUM banks;
    the bias row enters via a K=1 (c x ones) matmul per bank. The
    phase-B chunk-0 stream DMA is interleaved into the M trigger
    sequence so it lands just before phase A's compute finishes.
  * phase B: x^T streams in 512-column chunks (double-buffered).
    Softmax is ONLINE: per (i, chunk) a running negated max is
    maintained on DVE straight out of PSUM, and ACT applies
    exp(PSUM - runmax) directly into bf16 E with accum_out collecting
    per-chunk partial sums. No S buffer exists.
  * after the last chunk per i: c_k = exp(m_k - m_final), Z = sum_k
    z_k c_k, g_k = c_k / Z. The g_k become 32 per-(i,chunk) DIAGONAL
    matrices (bf16), built i-grouped alternating DVE/ACT; phase C
    "transposes" are plain matmuls E_tile @ diag(g) so the softmax
    normalization and running-max corrections ride the mandatory
    transpose for free.
  * phase C: x (bf16) is fully resident in SBUF (8MB, loaded on the
    sync ring behind the phase-B stream; its buffer aliases only the
    phase-A weight pool so the triggers fire as soon as the ring
    drains). Passes over i-tiles {0,1}/{2}/{3}: pass 0 needs only the
    first two diags so its transposes start while phase B's last
    matmuls still run; the last pass leaves just 512KB of output for
    the tail. A single shared PSUM transpose pool avoids cross-pass
    bank WARs (pass 1 lands on never-used banks). Output copy/DMA
    pairs alternate DVE/ACT engines and sync/scalar DMA rings.
"""

import os
from contextlib import ExitStack

import numpy as np
import ml_dtypes

import concourse.bass as bass
import concourse.mybir as mybir
import concourse.tile as tile
from concourse import bacc
from concourse.bass_utils import run_bass_kernel_spmd
from concourse.masks import make_identity

N, D = 4096, 1024
NCORES = 8
R = N // NCORES  # 512 query rows per core
PT = 128  # partition tile
EC = D // PT  # 8 contraction chunks of the model dim
IT = R // PT  # 4 query tiles per core
JC = N // 512  # 8 key chunks of 512
JT = N // PT  # 32 key tiles of 128

F32 = mybir.dt.float32
F32R = mybir.dt.float32r
BF16 = mybir.dt.bfloat16
AX = mybir.AxisListType
AF = mybir.ActivationFunctionType
ALU = mybir.AluOpType

PASSES = ((0, 1), (2,), (3,))


def _emit(nc: bass.Bass, tc: tile.TileContext, aps: dict):
    xTb, xTs, mw, cw, xb, out = (
        aps["xTb"], aps["xTs"], aps["mw"], aps["cw"],
        aps["xb"], aps["out"],
    )

    with ExitStack() as big:
        persist = big.enter_context(tc.tile_pool(name="persist", bufs=1))

        ident = persist.tile([PT, PT], BF16)
        make_identity(nc, ident)
        c_sb = persist.tile([PT, EC], F32)

        tT_sb = persist.tile([PT, EC, R], F32R)
        E_bf = [persist.tile([PT, N], BF16, name=f"E{i}") for i in range(IT)]
        nmk = [persist.tile([PT, JC], F32, name=f"nmk{i}") for i in range(IT)]
        tmx = [persist.tile([PT, JC], F32, name=f"tmx{i}") for i in range(IT)]
        zpart = [persist.tile([PT, JC], F32, name=f"zp{i}") for i in range(IT)]
        ck = [persist.tile([PT, JC], F32, name=f"ck{i}") for i in range(IT)]
        gk = [persist.tile([PT, JC], F32, name=f"gk{i}") for i in range(IT)]
        zsum = [persist.tile([PT, 1], F32, name=f"z{i}") for i in range(IT)]
        rz = [persist.tile([PT, 1], F32, name=f"rz{i}") for i in range(IT)]
        diag = persist.tile([PT, IT, JC, PT], BF16)

        # opened before wpool so its addresses never overlap the weights;
        # the early stream triggers can then issue during phase A.
        xtpool = big.enter_context(tc.tile_pool(name="xtpool", bufs=4))
        xtjs = {}
        for j in range(1, JC):
            xtjs[j] = xtpool.tile([PT, EC, 512], F32R, tag="xtj", name="xtj")

        # xs in its own pool (opened after xtpool so pools unwind LIFO):
        # stream position 0 of phase B reads it directly - each core's query
        # slice IS its own key chunk - so it is only released after that;
        # the xb buffer then aliases it + wpool.
        xspool_cm = tc.tile_pool(name="xspool", bufs=1)
        xspool = xspool_cm.__enter__()
        xts_sb = xspool.tile([PT, EC, R], F32R)

        # ---- Phase A: tT = M^T.xs^T + c  (transposed layout)
        with ExitStack() as pha:
            wpool = pha.enter_context(tc.tile_pool(name="wpool", bufs=1))
            apsum = pha.enter_context(tc.tile_pool(name="apsum", bufs=1, space="PSUM"))

            m_sb = wpool.tile([PT, EC, D], F32R)

            m_r = mw.rearrange("(t p) d -> p t d", p=PT)
            xTs_r = xTs.rearrange("(t p) i -> p t i", p=PT)
            # M rides the sync HWDGE ring, xs + bias the scalar ring: the
            # trigger FIFOs are independent and the SDMA engines round-robin
            # between them. Chunk 0 of the phase-B stream is slotted in
            # before the last two M chunks: phase A's compute tail covers it.
            nc.sync.dma_start(m_sb[:, 0, 0:PT], m_r[:, 0, 0:PT])
            nc.scalar.dma_start(xts_sb[:, 0, 0:256], xTs_r[:, 0, 0:256])
            nc.scalar.dma_start(xts_sb[:, 0, 256:512], xTs_r[:, 0, 256:512])
            nc.sync.dma_start(m_sb[:, 0, PT:D], m_r[:, 0, PT:D])
            nc.scalar.dma_start(xts_sb[:, 1, :], xTs_r[:, 1, :])
            nc.scalar.dma_start(c_sb, cw)
            for e in range(1, EC):
                nc.sync.dma_start(m_sb[:, e, :], m_r[:, e, :])
            for e in range(2, EC):
                nc.scalar.dma_start(xts_sb[:, e, :], xTs_r[:, e, :])

            tps = [
                apsum.tile([PT, R], F32, tag=f"tp{d}", name=f"tp{d}")
                for d in range(EC)
            ]
            # PE warm-up while the first DMA chunks land: back-to-back tiny
            # matmuls keep the activity monitor busy so the PE clock is at
            # full p-state when the real work starts. Results are clobbered
            # by the first start=True matmul into the same bank.
            for _ in range(28):
                nc.tensor.matmul(
                    tps[0][:, 0:PT], ident, ident, start=True, stop=True
                )
            for e in range(EC):
                for d in range(EC):
                    nc.tensor.matmul(
                        tps[d],
                        m_sb[:, e, d * PT : (d + 1) * PT],
                        xts_sb[:, e, :],
                        start=(e == 0),
                        stop=(e == EC - 1),
                    )
            for d in range(EC):
                # bias folded into the PSUM->SBUF copy: tT[d_blk,:] += c[d_blk]
                if d % 2 == 0:
                    nc.vector.tensor_scalar_add(
                        tT_sb[:, d, :], tps[d], c_sb[:, d : d + 1]
                    )
                else:
                    nc.scalar.activation(
                        tT_sb[:, d, :], tps[d], func=AF.Identity,
                        bias=c_sb[:, d : d + 1],
                    )

        # ---- Phase B: S chunks in PSUM + online softmax straight to E.
        def softmax_step(ps, i, j):
            if j == 0:
                nc.vector.reduce_max(
                    out=nmk[i][:, 0:1], in_=ps, axis=AX.X, negate=True
                )
            else:
                nc.vector.reduce_max(
                    out=tmx[i][:, j : j + 1], in_=ps, axis=AX.X, negate=True
                )
                nc.vector.tensor_tensor(
                    out=nmk[i][:, j : j + 1],
                    in0=nmk[i][:, j - 1 : j],
                    in1=tmx[i][:, j : j + 1],
                    op=ALU.min,
                )
            nc.scalar.activation(
                out=E_bf[i][:, j * 512 : (j + 1) * 512],
                in_=ps,
                func=AF.Exp,
                bias=nmk[i][:, j : j + 1],
                scale=1.0,
                accum_out=zpart[i][:, j : j + 1],
            )

        def finalize_pair(ia, ib):
            # c_k = exp(m_k - m_last), Z = sum z_k c_k, g = c_k/Z; then the
            # per-chunk diag(g) tiles, k-ordered round-robin across DVE/ACT
            # so both i-tiles' early-k diags finish first, in parallel.
            for i in (ia, ib):
                nc.scalar.activation(
                    out=ck[i],
                    in_=nmk[i],
                    func=AF.Exp,
                    bias=nmk[i][:, JC - 1 : JC],
                    scale=-1.0,
                )
            for i in (ia, ib):
                nc.vector.tensor_tensor(
                    out=gk[i], in0=zpart[i], in1=ck[i], op=ALU.mult
                )
            for i in (ia, ib):
                nc.vector.reduce_sum(out=zsum[i], in_=gk[i], axis=AX.X)
            for i in (ia, ib):
                nc.vector.reciprocal(rz[i], zsum[i])
            for i in (ia, ib):
                nc.vector.tensor_scalar_mul(gk[i], ck[i], rz[i])
            for k in range(JC):
                dve_i = ia if k % 2 == 0 else ib
                act_i = ib if k % 2 == 0 else ia
                nc.vector.tensor_scalar_mul(
                    diag[:, dve_i, k, :], ident, gk[dve_i][:, k : k + 1]
                )
                nc.scalar.activation(
                    diag[:, act_i, k, :],
                    ident,
                    func=AF.Copy,
                    scale=gk[act_i][:, k : k + 1],
                )

        bpend = []
        with ExitStack() as phb:
            spsum = phb.enter_context(tc.tile_pool(name="spsum", bufs=4, space="PSUM"))
            padpool = phb.enter_context(
                tc.tile_pool(name="padpool", bufs=1, space="PSUM")
            )
            def mm_group(ps, i, xtj):
                for d in range(EC):
                    nc.tensor.matmul(
                        ps,
                        tT_sb[:, d, i * PT : (i + 1) * PT],
                        xtj[:, d, :],
                        start=(d == 0),
                        stop=(d == EC - 1),
                    )

            for j in range(JC - 2):
                xtj = xts_sb if j == 0 else xtjs[j]
                if j > 0:
                    nc.sync.dma_start(xtj, xTb[j])
                for i in range(IT):
                    ps = spsum.tile([PT, 512], F32, tag="Sp", name="Sp")
                    mm_group(ps, i, xtj)
                    softmax_step(ps, i, j)
                if j == 0:
                    xspool_cm.__exit__(None, None, None)

            # Chunks 6-7 are processed i0/i1-first so their finalize chain
            # (which gates phase C's first transposes) completes ~2 PE groups
            # before B's compute ends. i3's last group goes to a dedicated
            # PSUM bank so its deferred exp gates nothing in phase C.
            nc.sync.dma_start(xtjs[JC - 2], xTb[JC - 2])
            nc.sync.dma_start(xtjs[JC - 1], xTb[JC - 1])
            xt6, xt7 = xtjs[JC - 2], xtjs[JC - 1]

            def subchain(i):
                # gk = ck * (1/Z); first two diag tiles built inline on DVE.
                nc.vector.tensor_tensor(
                    out=gk[i], in0=zpart[i], in1=ck[i], op=ALU.mult
                )
                nc.vector.reduce_sum(out=zsum[i], in_=gk[i], axis=AX.X)
                nc.vector.reciprocal(rz[i], zsum[i])
                nc.vector.tensor_scalar_mul(gk[i], ck[i], rz[i])
                for k in (0, 1):
                    nc.vector.tensor_scalar_mul(
                        diag[:, i, k, :], ident, gk[i][:, k : k + 1]
                    )

            for i in (0, 1):
                ps = spsum.tile([PT, 512], F32, tag="Sp", name="Sp")
                mm_group(ps, i, xt6)
                softmax_step(ps, i, JC - 2)
            ps70 = spsum.tile([PT, 512], F32, tag="Sp", name="Sp")
            mm_group(ps70, 0, xt7)
            ps71 = spsum.tile([PT, 512], F32, tag="Sp", name="Sp")
            mm_group(ps71, 1, xt7)
            softmax_step(ps70, 0, JC - 1)
            nc.scalar.activation(
                out=ck[0], in_=nmk[0], func=AF.Exp,
                bias=nmk[0][:, JC - 1 : JC], scale=-1.0,
            )
            softmax_step(ps71, 1, JC - 1)
            nc.scalar.activation(
                out=ck[1], in_=nmk[1], func=AF.Exp,
                bias=nmk[1][:, JC - 1 : JC], scale=-1.0,
            )
            subchain(0)
            subchain(1)
            for i in (2, 3):
                ps = spsum.tile([PT, 512], F32, tag="Sp", name="Sp")
                mm_group(ps, i, xt6)
                softmax_step(ps, i, JC - 2)
            ps72 = spsum.tile([PT, 512], F32, tag="Sp", name="Sp")
            mm_group(ps72, 2, xt7)
            softmax_step(ps72, 2, JC - 1)
            # pad tiles reserve banks 4-6 (never written) so the final i3
            # group lands on bank 7, which phase C never reallocates; its
            # exp can then be deferred into phase C's queues safely.
            for pb in range(3):
                padpool.tile([PT, 512], F32, tag=f"pad{pb}", name=f"pad{pb}")
            ps73 = padpool.tile([PT, 512], F32, tag="Spz", name="Spz")
            mm_group(ps73, 3, xt7)

            def red3min3():
                nc.vector.reduce_max(
                    out=tmx[3][:, JC - 1 : JC], in_=ps73, axis=AX.X, negate=True
                )
                nc.vector.tensor_tensor(
                    out=nmk[3][:, JC - 1 : JC], in0=nmk[3][:, JC - 2 : JC - 1],
                    in1=tmx[3][:, JC - 1 : JC], op=ALU.min,
                )

            def exp3():
                nc.scalar.activation(
                    out=E_bf[3][:, (JC - 1) * 512 : JC * 512],
                    in_=ps73, func=AF.Exp,
                    bias=nmk[3][:, JC - 1 : JC], scale=1.0,
                    accum_out=zpart[3][:, JC - 1 : JC],
                )

            def dgk01(k):
                def emit():
                    nc.vector.tensor_scalar_mul(
                        diag[:, 0, k, :], ident, gk[0][:, k : k + 1]
                    )
                    nc.scalar.activation(
                        diag[:, 1, k, :], ident, func=AF.Copy,
                        scale=gk[1][:, k : k + 1],
                    )
                return emit

            def fin23a():
                for i in (2, 3):
                    nc.scalar.activation(
                        out=ck[i], in_=nmk[i], func=AF.Exp,
                        bias=nmk[i][:, JC - 1 : JC], scale=-1.0,
                    )
                for i in (2, 3):
                    nc.vector.tensor_tensor(
                        out=gk[i], in0=zpart[i], in1=ck[i], op=ALU.mult
                    )
                for i in (2, 3):
                    nc.vector.reduce_sum(out=zsum[i], in_=gk[i], axis=AX.X)
                for i in (2, 3):
                    nc.vector.reciprocal(rz[i], zsum[i])
                for i in (2, 3):
                    nc.vector.tensor_scalar_mul(gk[i], ck[i], rz[i])

            def dg23(k0, k1):
                def emit():
                    for k in range(k0, k1):
                        nc.vector.tensor_scalar_mul(
                            diag[:, 2, k, :], ident, gk[2][:, k : k + 1]
                        )
                        nc.scalar.activation(
                            diag[:, 3, k, :], ident, func=AF.Copy,
                            scale=gk[3][:, k : k + 1],
                        )
                return emit

            bpend.append(red3min3)
            bpend.append(exp3)
            for k in range(2, JC):
                bpend.append(dgk01(k))
            bpend.append(fin23a)
            bpend.append(dg23(0, 4))
            bpend.append(dg23(4, JC))

        # ---- Phase C: out = P @ x with x fully resident in SBUF.
        # xb reuses the phase-A weight pool's address range; its triggers sit
        # on the sync ring behind the phase-B stream.
        xbpool = big.enter_context(tc.tile_pool(name="xbpool", bufs=1))
        xb_sb = xbpool.tile([PT, JT, D], BF16)
        nc.sync.dma_start(xb_sb[:, 0 : JT // 2, :], xb[:, 0 : JT // 2, :])
        nc.sync.dma_start(xb_sb[:, JT // 2 : JT, :], xb[:, JT // 2 : JT, :])
        etpool = big.enter_context(tc.tile_pool(name="etpool", bufs=3))
        ocopy = big.enter_context(tc.tile_pool(name="ocopy", bufs=4))
        # opsum allocated before tpsum: oacc lands on the banks whose phase-B
        # WARs clear first (their mms only start LOOKP pairs in), tpsum gets
        # the late-released + fresh banks so the first transposes don't wait
        # on i2/i3's last exps.
        opsum = big.enter_context(tc.tile_pool(name="opsum", bufs=1, space="PSUM"))
        tpsum = big.enter_context(tc.tile_pool(name="tpsum", bufs=3, space="PSUM"))
        # accumulators are shared by both passes (keyed by position kp);
        # pass 1's first start=True matmuls depend on pass 0's drain copies,
        # which are emitted interleaved into pass 1's first pairs.
        oacc = {
            (kp, dn): opsum.tile([PT, 512], F32, tag=f"o{kp}_{dn}", name=f"o{kp}_{dn}")
            for kp in range(2)
            for dn in range(2)
        }

        def drain_item(kp, dn, i):
            def emit():
                ot = ocopy.tile([PT, 512], F32, tag="ot", name="ot")
                if dn == 0:
                    nc.vector.tensor_copy(ot, oacc[(kp, dn)])
                    nc.sync.dma_start(out[i * PT : (i + 1) * PT, 0:512], ot)
                else:
                    nc.scalar.activation(ot, oacc[(kp, dn)], func=AF.Copy)
                    nc.scalar.dma_start(out[i * PT : (i + 1) * PT, 512:1024], ot)

            return emit

        NP = JT // 2  # jt pairs
        LOOKP = 2
        pending = bpend
        for pi, ii in enumerate(((0, 1), (2, 3))):
            ets = {}
            for pv in range(NP + LOOKP):
                if pv < NP:
                    # "transpose" = E_tile.T @ diag(g): per-row softmax scale
                    # applied for free by the mandatory transpose. Two jt per
                    # pst bank -> one PSUM->SBUF copy per pair.
                    pst = tpsum.tile([PT, 512], F32, tag="tp", name="pst")
                    for kp, i in enumerate(ii):
                        for s in range(2):
                            jt = 2 * pv + s
                            nc.tensor.matmul(
                                pst[:, (2 * kp + s) * PT : (2 * kp + s + 1) * PT],
                                E_bf[i][:, jt * PT : (jt + 1) * PT],
                                diag[:, i, jt // 4, :],
                                start=True,
                                stop=True,
                                skip_group_check=True,
                            )
                    et = etpool.tile([PT, 512], BF16, tag="et", name="et")
                    if pv % 2 == 0:
                        nc.vector.tensor_copy(et, pst)
                    else:
                        nc.scalar.activation(et, pst, func=AF.Copy)
                    ets[pv % 3] = et
                    # pops start at pv 1 so pair 0's et copy isn't queued
                    # behind the deferred DVE work; two per pair so all of
                    # the previous pass's drains are emitted before this
                    # pass's first start=True matmuls reuse the banks.
                    if pv >= 1:
                        for _ in range(2):
                            if pending:
                                pending.pop(0)()
                if pv >= LOOKP:
                    p = pv - LOOKP
                    for s in range(2):
                        jt = 2 * p + s
                        for kp, i in enumerate(ii):
                            for dn in range(2):
                                nc.tensor.matmul(
                                    oacc[(kp, dn)],
                                    ets[p % 3][
                                        :, (2 * kp + s) * PT : (2 * kp + s + 1) * PT
                                    ],
                                    xb_sb[:, jt, dn * 512 : (dn + 1) * 512],
                                    start=(jt == 0),
                                    stop=(jt == JT - 1),
                                )
            drains = [
                drain_item(kp, dn, i)
                for kp, i in enumerate(ii)
                for dn in range(2)
            ]
            if pi == 0:
                pending = drains
            else:
                for d in drains:
                    d()


def build():
    nc = bacc.Bacc(
        "TRN2",
        target_bir_lowering=False,
        debug=False,
        enable_asserts=False,
        num_devices=NCORES,
    )
    aps = {
        "xTb": nc.dram_tensor("xTb", [JC, PT, EC, 512], F32R, kind="ExternalInput").ap(),
        "xTs": nc.dram_tensor("xTs", [D, R], F32R, kind="ExternalInput").ap(),
        "mw": nc.dram_tensor("mw", [D, D], F32R, kind="ExternalInput").ap(),
        "cw": nc.dram_tensor("cw", [PT, EC], F32, kind="ExternalInput").ap(),
        "xb": nc.dram_tensor("xb", [PT, JT, D], BF16, kind="ExternalInput").ap(),
        "out": nc.dram_tensor("out", [R, D], F32, kind="ExternalOutput").ap(),
    }
    with tile.TileContext(nc) as tc:
        _emit(nc, tc, aps)
    nc.compile()
    return nc


_NC_CACHE = None
LAST_RESULTS = None


def _get_nc():
    global _NC_CACHE
    if _NC_CACHE is None:
        _NC_CACHE = build()
    return _NC_CACHE


def make_in_maps(x, Wq, bq, Wk):
    x = np.ascontiguousarray(np.asarray(x, dtype=np.float32))
    xT = np.ascontiguousarray(x.T)
    # xTb[j, p, e, n] = xT[e*128 + p, j*512 + n]: per-(j,p) contiguous 16KB
    # blocks so the phase-B stream DMAs at full descriptor size.
    xTb = np.ascontiguousarray(
        xT.reshape(EC, PT, JC, 512).transpose(2, 1, 0, 3)
    )
    wk64 = np.asarray(Wk, dtype=np.float64)
    mw = np.ascontiguousarray(
        (np.asarray(Wq, dtype=np.float64).T @ wk64).astype(np.float32)
    )
    # cw[p, e] = c[e*128 + p]: per-partition bias column for the tT copies.
    cw = np.ascontiguousarray(
        (np.asarray(bq, dtype=np.float64) @ wk64)
        .astype(np.float32)
        .reshape(EC, PT)
        .T
    )
    xb = x.astype(ml_dtypes.bfloat16)
    in_maps = []
    for c in range(NCORES):
        # Each core processes key chunks in rotated order [c, c+1, ..]: its
        # own query slice xTs doubles as stream position 0 (already in SBUF
        # when phase B starts), so xTb and xb are rotated to match. The
        # rotation permutes softmax terms and P@x rows consistently; the
        # output rows (queries) are unaffected.
        in_maps.append(
            {
                "xTb": np.ascontiguousarray(
                    np.concatenate([xTb[c:], xTb[:c]], axis=0)
                ),
                "xTs": np.ascontiguousarray(xT[:, c * R : (c + 1) * R]),
                "mw": mw,
                "cw": cw,
                "xb": np.ascontiguousarray(
                    np.roll(xb, -512 * c, axis=0)
                    .reshape(JT, PT, D)
                    .transpose(1, 0, 2)
                ),
            }
        )
    return in_maps


def kernel(x, Wq, bq, Wk, bk):
    # bk only shifts each score row by a constant, which softmax cancels.
    del bk
    in_maps = make_in_maps(x, Wq, bq, Wk)
    nc = _get_nc()
    kwargs = {}
    if os.environ.get("K_TRACE_DIR"):
        import tempfile

        kwargs["tmpdir"] = tempfile.mkdtemp(dir=os.environ["K_TRACE_DIR"])
    res = run_bass_kernel_spmd(nc, in_maps, core_ids=list(range(NCORES)), **kwargs)
    global LAST_RESULTS
    LAST_RESULTS = res
    return np.concatenate(
        [np.asarray(res.results[c]["out"], dtype=np.float32) for c in range(NCORES)],
        axis=0,
    )
